# revision 24
# baseline (speedup 1.0000x reference)
# DeepSet Trainium2 kernel.
#
# Strategy: events are sorted by jet-count n (2..10) on the host and
# round-robin sharded across 8 cores into per-group slots of capacity cap_g
# (multiple of 8, exact-packed). Within a group every event has exactly n=g
# valid jets, so all masks, pair structures and aggregation counts are
# compile-time constants.
#
# Math folding (host, O(params)):
#   every Dense+BN+relu block becomes relu(h @ W' + b') with W', b' folded.
#   MLP2 layer 1 uses the z-trick: y1 = relu(z_i + z_j + t) with z = x @ Wz'.
#   t is folded into the y1 relu pass (tensor_scalar add-bias+max0).
#
# Device layout: feature-major [H=128 partitions, columns = slice*cap + b]
# per group, all activations bf16 (PE: 1 col/cycle), PSUM f32.
# Work distribution across engines:
#   PE:   all matmuls + Sum/Sumsq of pairs via PSUM-accumulating identity mms
#   Act:  x1/x2/z/x PSUM evacs (relu+bias / copy), y2 evac, xsq (Square)
#   DVE:  x-side sum/sumsq/max trees, y1 add (broadcast AP) + relu,
#         ysq (y3*y3), y-side max tree
#   Pool: y3 PSUM evac (tensor_scalar bias+relu)
# Mean/Var and the final [events, 4H] transpose are computed on the HOST
# from the 6 DMA'd feature-major aggregates (sum/sumsq/max per side).
import math
from contextlib import ExitStack

import numpy as np

import concourse.bass as bass
import concourse.bacc as bacc
import concourse.tile as tile
import concourse.mybir as mybir

f32 = mybir.dt.float32
bf16 = mybir.dt.bfloat16
AF = mybir.ActivationFunctionType
ALU = mybir.AluOpType

H = 128
FJ = 16


def pairs_of(g):
    return [(i, j) for i in range(g) for j in range(i + 1, g)]


# GPSIMD cannot access PSUM on TRN2 (verified: birverifier rejects it),
# and its 2-ALU tensor_scalar path is ~10x below its tensor_tensor rate
# (measured). So Pool only gets SBUF tensor_tensor work (squares, max L1).


def build_program(groups):
    """groups: list of (g, cap) with cap a multiple of 8, cap <= 512."""
    JC = sum(g * cap for g, cap in groups)
    EC = sum(cap for _, cap in groups)

    nc = bacc.Bacc("TRN2", target_bir_lowering=False, debug=False)

    jets_d = nc.dram_tensor("jets", [FJ, JC], bf16, kind="ExternalInput")
    w1_d = nc.dram_tensor("w1", [FJ, H], bf16, kind="ExternalInput")
    w2_d = nc.dram_tensor("w2", [H, H], bf16, kind="ExternalInput")
    w3_d = nc.dram_tensor("w3", [H, H], bf16, kind="ExternalInput")
    wz_d = nc.dram_tensor("wz", [H, H], bf16, kind="ExternalInput")
    w4_d = nc.dram_tensor("w4", [H, H], bf16, kind="ExternalInput")
    w5_d = nc.dram_tensor("w5", [H, H], bf16, kind="ExternalInput")
    identp_d = nc.dram_tensor("identp", [H, H], bf16, kind="ExternalInput")
    # bias vector cols: 0..5 = b1, b2, b3, t(=bz), b4, b5
    bv_d = nc.dram_tensor("bvec", [H, 8], f32, kind="ExternalInput")
    # per group: 6 aggregates [H, cap] each, packed [sx qx mx sy qy my]
    out6_d = nc.dram_tensor("out6", [H, 6 * EC], f32, kind="ExternalOutput")

    with tile.TileContext(nc) as tc, ExitStack() as ctx:
        consts = ctx.enter_context(tc.tile_pool(name="consts", bufs=1))
        jin = ctx.enter_context(tc.tile_pool(name="jin", bufs=2))
        x12 = ctx.enter_context(tc.tile_pool(name="x12", bufs=2))
        bigx = ctx.enter_context(tc.tile_pool(name="bigx", bufs=2))
        bigy = ctx.enter_context(tc.tile_pool(name="bigy", bufs=2))
        scr = ctx.enter_context(tc.tile_pool(name="scr", bufs=2))
        mxp = ctx.enter_context(tc.tile_pool(name="mxp", bufs=2))
        aggs = ctx.enter_context(tc.tile_pool(name="aggs", bufs=2))
        mm = ctx.enter_context(tc.tile_pool(name="mm", bufs=2, space="PSUM"))
        acc = ctx.enter_context(tc.tile_pool(name="acc", bufs=2, space="PSUM"))

        def const_tile(name, dram, shape, dt):
            t = consts.tile(shape, dt, tag=name)
            nc.sync.dma_start(t[:], dram.ap())
            return t

        w1t = const_tile("w1", w1_d, [FJ, H], bf16)
        w2t = const_tile("w2", w2_d, [H, H], bf16)
        w3t = const_tile("w3", w3_d, [H, H], bf16)
        wzt = const_tile("wz", wz_d, [H, H], bf16)
        w4t = const_tile("w4", w4_d, [H, H], bf16)
        w5t = const_tile("w5", w5_d, [H, H], bf16)
        ip_t = const_tile("ip", identp_d, [H, H], bf16)
        bv = const_tile("bv", bv_d, [H, 8], f32)

        def r3(ap, k):
            return ap.rearrange("p (k c) -> p k c", k=k)

        jets_off = 0
        ev_off = 0
        pending_tail = [None]
        for gi, (g, cap) in enumerate(groups):
            assert cap % 8 == 0 and cap <= 512
            JCg = g * cap
            prs = pairs_of(g)
            PG = len(prs)

            jt = jin.tile([FJ, JCg], bf16, tag="jt")
            nc.sync.dma_start(jt[:], jets_d.ap()[:, jets_off : jets_off + JCg])

            # ---- jets side: 4 layers, layer-major 1024-col chunks.
            def layer(dst, wt, src, width, evac):
                for c0 in range(0, width, 1024):
                    w = min(1024, width - c0)
                    ps = mm.tile([H, 1024], f32, tag="mm")
                    for s0 in range(0, w, 512):
                        sw = min(512, w - s0)
                        nc.tensor.matmul(ps[:, s0 : s0 + sw], wt[:],
                                         src[:, c0 + s0 : c0 + s0 + sw],
                                         start=True, stop=True)
                    evac(dst[:, c0 : c0 + w], ps[:, :w])

            def act_relu(bias_col):
                def f(dst, ps):
                    nc.scalar.activation(dst, ps, AF.Relu,
                                         bias=bv[:, bias_col : bias_col + 1])
                return f

            def act_copy(dst, ps):
                nc.scalar.copy(dst, ps)

            x1 = x12.tile([H, JCg], bf16, tag="x1")
            layer(x1, w1t, jt, JCg, act_relu(0))
            x2 = x12.tile([H, JCg], bf16, tag="x2")
            layer(x2, w2t, x1, JCg, act_relu(1))
            x = bigx.tile([H, JCg], bf16, tag="x")
            layer(x, w3t, x2, JCg, act_relu(2))
            z = bigx.tile([H, JCg], bf16, tag="z")
            layer(z, wzt, x, JCg, act_copy)

            # Tail of the previous group (its last-chunk sums, max tree,
            # accumulator evacs and output DMA) is emitted here so its PE /
            # DVE / Scalar work overlaps this group's jets layers.
            if pending_tail[0] is not None:
                pending_tail[0]()
                pending_tail[0] = None

            xsq = bigx.tile([H, JCg], bf16, tag="xsq")
            nc.gpsimd.tensor_tensor(xsq[:], x[:], x[:], ALU.mult)

            agg6 = aggs.tile([H, 6 * cap], f32, tag="agg6")

            # ---- x-side trees on DVE (sum exact-halving, max overlap-halving)
            def sum_tree(src_tile, nslices, dst_f32):
                m, cur, off = nslices, src_tile, 0
                if m == 1:
                    nc.vector.tensor_copy(dst_f32, cur[:, 0:cap])
                    return
                while m > 1:
                    k2 = m // 2
                    if k2 == 1:
                        nxt = dst_f32
                    else:
                        nxt = mxp.tile([H, k2 * cap], bf16, tag="xt")
                    nc.vector.tensor_tensor(
                        r3(nxt[:, 0 : k2 * cap], k2),
                        r3(cur[:, off : off + k2 * cap], k2),
                        r3(cur[:, off + k2 * cap : off + 2 * k2 * cap], k2),
                        ALU.add)
                    if m % 2:
                        nc.vector.tensor_add(
                            nxt[:, 0:cap], nxt[:, 0:cap],
                            cur[:, off + (m - 1) * cap : off + m * cap])
                    cur, off, m = nxt, 0, k2

            def max_tree(src_tile, nslices, dst_f32, tag, l1_pool=False,
                         cap=cap):
                m, cur, off = nslices, src_tile, 0
                if m == 1:
                    nc.vector.tensor_copy(dst_f32, src_tile[:, 0:cap])
                    return
                first = True
                while m > 1:
                    k2 = (m + 1) // 2
                    if k2 == 1:
                        nxt = dst_f32
                    else:
                        nxt = mxp.tile([H, k2 * cap], bf16, tag=tag)
                    a0 = cur[:, off : off + k2 * cap]
                    a1 = cur[:, off + (m - k2) * cap : off + m * cap]
                    nc.vector.tensor_tensor(r3(nxt[:, 0 : k2 * cap], k2),
                                            r3(a0, k2), r3(a1, k2), ALU.max)
                    cur, off, m = nxt, 0, k2
                    first = False

            sum_tree(x, g, agg6[:, 0:cap])
            sum_tree(xsq, g, agg6[:, cap : 2 * cap])
            max_tree(x, g, agg6[:, 2 * cap : 3 * cap], "xt")

            # ---- pairs side: SC-slice chunks, lag-1 ysq+sums.
            SC = max(1, 2048 // cap)
            PCH = SC * cap
            y3 = bigy.tile([H, PG * cap], bf16, tag="y3")
            a_sy = acc.tile([H, cap], f32, tag="a_sy")
            a_qy = acc.tile([H, cap], f32, tag="a_qy")

            def ysq_sums(p0, k, cap=cap, PCH=PCH, y3=y3, PG=PG,
                         a_sy=a_sy, a_qy=a_qy):
                w = k * cap
                ysq = scr.tile([H, PCH], bf16, tag="ysq")
                nc.gpsimd.tensor_tensor(ysq[:, :w],
                                        y3[:, p0 * cap : p0 * cap + w],
                                        y3[:, p0 * cap : p0 * cap + w],
                                        ALU.mult)
                for s in range(k):
                    sl = p0 + s
                    nc.tensor.matmul(a_sy[:, :cap], ip_t[:],
                                     y3[:, sl * cap : (sl + 1) * cap],
                                     start=(sl == 0), stop=(sl == PG - 1))
                for s in range(k):
                    sl = p0 + s
                    nc.tensor.matmul(a_qy[:, :cap], ip_t[:],
                                     ysq[:, s * cap : (s + 1) * cap],
                                     start=(sl == 0), stop=(sl == PG - 1))

            prev = None
            for p0 in range(0, PG, SC):
                k = min(SC, PG - p0)
                w = k * cap
                y1 = scr.tile([H, PCH], bf16, tag="y1")
                s = 0
                while s < k:
                    i = prs[p0 + s][0]
                    r = 1
                    while s + r < k and prs[p0 + s + r][0] == i:
                        r += 1
                    j0 = prs[p0 + s][1]
                    nc.vector.tensor_tensor(
                        r3(y1[:, s * cap : (s + r) * cap], r),
                        r3(z[:, i * cap : (i + 1) * cap], 1).broadcast_to(
                            [H, r, cap]),
                        r3(z[:, j0 * cap : (j0 + r) * cap], r),
                        ALU.add)
                    s += r
                nc.vector.tensor_scalar(y1[:, :w], y1[:, :w], bv[:, 3:4],
                                        0.0, ALU.add, ALU.max)
                y2 = scr.tile([H, PCH], bf16, tag="y2")
                for n0 in range(0, w, 1024):
                    cw = min(1024, w - n0)
                    ps = mm.tile([H, 1024], f32, tag="mm")
                    for s0 in range(0, cw, 512):
                        sw = min(512, cw - s0)
                        nc.tensor.matmul(ps[:, s0 : s0 + sw], w4t[:],
                                         y1[:, n0 + s0 : n0 + s0 + sw],
                                         start=True, stop=True)
                    nc.scalar.activation(y2[:, n0 : n0 + cw], ps[:, :cw],
                                         AF.Relu, bias=bv[:, 4:5])
                for n0 in range(0, w, 1024):
                    cw = min(1024, w - n0)
                    ps = mm.tile([H, 1024], f32, tag="mm")
                    for s0 in range(0, cw, 512):
                        sw = min(512, cw - s0)
                        nc.tensor.matmul(ps[:, s0 : s0 + sw], w5t[:],
                                         y2[:, n0 + s0 : n0 + s0 + sw],
                                         start=True, stop=True)
                    dst = y3[:, p0 * cap + n0 : p0 * cap + n0 + cw]
                    nc.vector.tensor_scalar(dst, ps[:, :cw], bv[:, 5:6],
                                            0.0, ALU.add, ALU.max)
                if prev is not None:
                    ysq_sums(*prev)
                prev = (p0, k)

            def tail(prev=prev, y3=y3, PG=PG, cap=cap, agg6=agg6,
                     a_sy=a_sy, a_qy=a_qy, ev_off=ev_off, max_tree=max_tree,
                     ysq_sums=ysq_sums):
                ysq_sums(*prev)
                max_tree(y3, PG, agg6[:, 5 * cap : 6 * cap], "yt",
                         l1_pool=True)
                nc.scalar.copy(agg6[:, 3 * cap : 4 * cap], a_sy[:, :cap])
                nc.scalar.copy(agg6[:, 4 * cap : 5 * cap], a_qy[:, :cap])
                nc.sync.dma_start(
                    out6_d.ap()[:, 6 * ev_off : 6 * ev_off + 6 * cap],
                    agg6[:])

            pending_tail[0] = tail

            jets_off += JCg
            ev_off += cap
        pending_tail[0]()

    nc.compile()
    return nc


# ---------------- host-side math ----------------

BN_EPS = 1e-3


def fold_params(inp):
    """Fold normalization + BN into per-layer (W, b). All numpy fp32."""
    mean_j = np.asarray(inp["mean_jets"], np.float32)
    std_j = np.asarray(inp["std_jets"], np.float32)
    w1f = np.asarray(inp["w1_first"], np.float32)
    w1r = np.asarray(inp["w1_rest"], np.float32)
    bn1 = np.asarray(inp["bn1"], np.float32)  # [3,4,H]: gamma, beta, mean, var
    w2f = np.asarray(inp["w2_first"], np.float32)
    w2r = np.asarray(inp["w2_rest"], np.float32)
    bn2 = np.asarray(inp["bn2"], np.float32)

    def bn_sb(row):
        gm, bt, mu, vv = row[0], row[1], row[2], row[3]
        s = gm / np.sqrt(vv + BN_EPS)
        return s.astype(np.float32), (bt - mu * s).astype(np.float32)

    s11, t11 = bn_sb(bn1[0]); s12, t12 = bn_sb(bn1[1]); s13, t13 = bn_sb(bn1[2])
    s21, t21 = bn_sb(bn2[0]); s22, t22 = bn_sb(bn2[1]); s23, t23 = bn_sb(bn2[2])

    A = w1f / std_j[:, None]
    c = -(mean_j / std_j) @ w1f
    return dict(
        W1=A * s11[None, :], b1=c * s11 + t11,
        W2=w1r[0] * s12[None, :], b2=t12,
        W3=w1r[1] * s13[None, :], b3=t13,
        Wz=w2f * s21[None, :], bz=t21,
        W4=w2r[0] * s22[None, :], b4=t22,
        W5=w2r[1] * s23[None, :], b5=t23,
    )


# ---------------- full kernel entry point ----------------

N_CORES = 8

_cache = {}
_TRACE = [False]
_LAST_RESULT = [None]


def _get_program(groups_key):
    if groups_key not in _cache:
        _cache[groups_key] = build_program(list(groups_key))
    return _cache[groups_key]


def _np_dt(dt):
    return mybir.dt.np(dt)


def _plan(n):
    """Returns (groups, slots): groups = [(g, cap)], slots[c][gi] =
    (padded index array, real count) for core c, group gi."""
    gs = []
    idx_by_g = {}
    for g in range(2, 11):
        idx = np.nonzero(n == g)[0]
        if len(idx):
            gs.append(g)
            idx_by_g[g] = idx
    stray = np.nonzero((n < 2) | (n > 10))[0]
    if len(stray):
        if not gs:
            gs.append(2)
            idx_by_g[2] = stray
        else:
            idx_by_g[gs[-1]] = np.concatenate([idx_by_g[gs[-1]], stray])
    groups = []
    slots = [[] for _ in range(N_CORES)]
    for g in gs:
        idx = idx_by_g[g]
        per_core = [idx[c::N_CORES] for c in range(N_CORES)]
        mx = max(len(p) for p in per_core)
        cap = max(8, ((mx + 7) // 8) * 8)
        groups.append((g, cap))
        fill = idx[0]
        for c in range(N_CORES):
            p = per_core[c]
            pad = np.full(cap, p[0] if len(p) else fill, dtype=np.int64)
            pad[: len(p)] = p
            slots[c].append((pad, len(p)))
    return groups, slots


def _pack_jets(jets, groups, slots_c):
    cols = []
    for (g, cap), (ids, _cnt) in zip(groups, slots_c):
        ev = jets[ids][:, :g, :]  # [cap, g, 16]
        cols.append(np.ascontiguousarray(ev.transpose(2, 1, 0)).reshape(
            FJ, g * cap))
    return np.concatenate(cols, axis=1).astype(_np_dt(bf16), copy=False)


def kernel(**inputs):
    from concourse.bass_utils import run_bass_kernel_spmd

    jets = np.asarray(inputs["inputs_jets"], dtype=np.float32)
    B = jets.shape[0]
    mask = (jets != 0.0).any(-1)
    n = mask.sum(-1).astype(np.int64)
    # compact valid jets to the front (no-op for the standard generator)
    if not np.array_equal(mask, np.arange(jets.shape[1])[None, :] < n[:, None]):
        order = np.argsort(~mask, axis=1, kind="stable")
        jets = np.take_along_axis(jets, order[:, :, None], axis=1)

    P = fold_params(inputs)
    groups, slots = _plan(n)
    nc = _get_program(tuple(groups))

    bvec = np.zeros((H, 8), np.float32)
    for i, k in enumerate(["b1", "b2", "b3", "bz", "b4", "b5"]):
        bvec[:, i] = P[k]
    ident = np.eye(H, dtype=np.float32)
    bnp = _np_dt(bf16)
    common = {
        "w1": P["W1"].astype(bnp), "w2": P["W2"].astype(bnp),
        "w3": P["W3"].astype(bnp), "wz": P["Wz"].astype(bnp),
        "w4": P["W4"].astype(bnp), "w5": P["W5"].astype(bnp),
        "identp": ident.astype(bnp), "bvec": bvec,
    }
    in_maps = []
    for c in range(N_CORES):
        m = dict(common)
        m["jets"] = _pack_jets(jets, groups, slots[c])
        in_maps.append(m)

    res = run_bass_kernel_spmd(nc, in_maps, core_ids=list(range(N_CORES)),
                               trace=_TRACE[0])
    _LAST_RESULT[0] = res

    agg_x = np.empty((B, 4 * H), np.float32)
    agg_y = np.empty((B, 4 * H), np.float32)
    for c in range(N_CORES):
        o6 = res.results[c]["out6"]  # [H, 6*EC] f32
        ev_off = 0
        for (g, cap), (ids, cnt) in zip(groups, slots[c]):
            blk = o6[:, 6 * ev_off : 6 * ev_off + 6 * cap]
            sx = blk[:, 0:cap].T[:cnt]
            qx = blk[:, cap : 2 * cap].T[:cnt]
            mx = blk[:, 2 * cap : 3 * cap].T[:cnt]
            sy = blk[:, 3 * cap : 4 * cap].T[:cnt]
            qy = blk[:, 4 * cap : 5 * cap].T[:cnt]
            my = blk[:, 5 * cap : 6 * cap].T[:cnt]
            ii = ids[:cnt]
            mean_x = sx / g
            mean_y = sy / (g * (g - 1) // 2)
            agg_x[ii] = np.concatenate(
                [sx, mx, mean_x, qx / g - mean_x * mean_x], axis=1)
            agg_y[ii] = np.concatenate(
                [sy, my, mean_y, qy / (g * (g - 1) // 2) - mean_y * mean_y],
                axis=1)
            ev_off += cap
    return agg_x, agg_y


# revision 32
# speedup vs baseline: 1.2680x; 1.2680x over previous
# DeepSet Trainium2 kernel.
#
# Strategy: events are sorted by jet-count n (2..10) on the host and
# round-robin sharded across 8 cores into per-group slots of capacity cap_g
# (multiple of 8, exact-packed). Within a group every event has exactly n=g
# valid jets, so all masks, pair structures and aggregation counts are
# compile-time constants.
#
# Math folding (host, O(params)):
#   every Dense+BN+relu block becomes relu(h @ W' + b') with W', b' folded.
#   MLP2 layer 1 uses the z-trick: y1 = relu(z_i + z_j + t) with z = x @ Wz'.
#   t is folded into the y1 relu pass (tensor_scalar add-bias+max0).
#
# Device layout: feature-major [H=128 partitions, columns = slice*cap + b]
# per group, all activations bf16 (PE: 1 col/cycle), PSUM f32.
# Work distribution across engines:
#   PE:   all matmuls + Sum/Sumsq of pairs via PSUM-accumulating identity mms
#   Act:  x1/x2/z/x PSUM evacs (relu+bias / copy), y2 evac, xsq (Square)
#   DVE:  x-side sum/sumsq/max trees, y1 add (broadcast AP) + relu,
#         ysq (y3*y3), y-side max tree
#   Pool: y3 PSUM evac (tensor_scalar bias+relu)
# Mean/Var and the final [events, 4H] transpose are computed on the HOST
# from the 6 DMA'd feature-major aggregates (sum/sumsq/max per side).
import math
from contextlib import ExitStack

import numpy as np

import concourse.bass as bass
import concourse.bacc as bacc
import concourse.tile as tile
import concourse.mybir as mybir

f32 = mybir.dt.float32
bf16 = mybir.dt.bfloat16
AF = mybir.ActivationFunctionType
ALU = mybir.AluOpType

H = 128
FJ = 16


def pairs_of(g):
    return [(i, j) for i in range(g) for j in range(i + 1, g)]


# GPSIMD findings (measured): cannot access PSUM (birverifier), has no
# max opcode, tensor_scalar runs ~12ns/col, and even its decent-rate
# tensor_tensor mult slows the Vector engine ~20-30% via SBUF port
# contention. Net negative everywhere -> unused.
# pow is rejected by the DVE ISA check (tensor_scalar_valid_ops).
USE_POW_SQUARE = False


def build_program(groups):
    """groups: list of (g, cap) with cap a multiple of 8, cap <= 512."""
    JC = sum(g * cap for g, cap in groups)
    EC = sum(cap for _, cap in groups)

    nc = bacc.Bacc("TRN2", target_bir_lowering=False, debug=False)

    jets_d = nc.dram_tensor("jets", [FJ, JC], bf16, kind="ExternalInput")
    w1_d = nc.dram_tensor("w1", [FJ, H], bf16, kind="ExternalInput")
    w2_d = nc.dram_tensor("w2", [H, H], bf16, kind="ExternalInput")
    w3_d = nc.dram_tensor("w3", [H, H], bf16, kind="ExternalInput")
    wz_d = nc.dram_tensor("wz", [H, H], bf16, kind="ExternalInput")
    w4_d = nc.dram_tensor("w4", [H, H], bf16, kind="ExternalInput")
    w5_d = nc.dram_tensor("w5", [H, H], bf16, kind="ExternalInput")
    identp_d = nc.dram_tensor("identp", [H, H], bf16, kind="ExternalInput")
    # bias vector cols: 0..5 = b1, b2, b3, t(=bz), b4, b5
    bv_d = nc.dram_tensor("bvec", [H, 8], f32, kind="ExternalInput")
    # per group: 6 aggregates [H, cap] each, packed [sx qx mx sy qy my]
    out6_d = nc.dram_tensor("out6", [H, 6 * EC], f32, kind="ExternalOutput")

    with tile.TileContext(nc) as tc, ExitStack() as ctx:
        consts = ctx.enter_context(tc.tile_pool(name="consts", bufs=1))
        jin = ctx.enter_context(tc.tile_pool(name="jin", bufs=2))
        x12 = ctx.enter_context(tc.tile_pool(name="x12", bufs=2))
        bigx = ctx.enter_context(tc.tile_pool(name="bigx", bufs=2))
        bigy = ctx.enter_context(tc.tile_pool(name="bigy", bufs=2))
        scr = ctx.enter_context(tc.tile_pool(name="scr", bufs=2))
        mxp = ctx.enter_context(tc.tile_pool(name="mxp", bufs=2))
        aggs = ctx.enter_context(tc.tile_pool(name="aggs", bufs=2))
        mm = ctx.enter_context(tc.tile_pool(name="mm", bufs=2, space="PSUM"))
        acc = ctx.enter_context(tc.tile_pool(name="acc", bufs=2, space="PSUM"))

        def const_tile(name, dram, shape, dt):
            t = consts.tile(shape, dt, tag=name)
            nc.sync.dma_start(t[:], dram.ap())
            return t

        w1t = const_tile("w1", w1_d, [FJ, H], bf16)
        w2t = const_tile("w2", w2_d, [H, H], bf16)
        w3t = const_tile("w3", w3_d, [H, H], bf16)
        wzt = const_tile("wz", wz_d, [H, H], bf16)
        w4t = const_tile("w4", w4_d, [H, H], bf16)
        w5t = const_tile("w5", w5_d, [H, H], bf16)
        ip_t = const_tile("ip", identp_d, [H, H], bf16)
        bv = const_tile("bv", bv_d, [H, 8], f32)

        def r3(ap, k):
            return ap.rearrange("p (k c) -> p k c", k=k)

        # square via DVE tensor_scalar pow: hits the 4x bf16 perf mode
        # (tensor_tensor mult only reaches ~1 elem/cycle). Inputs are
        # relu outputs (>= 0) so pow is safe.
        def square(dst, src):
            if USE_POW_SQUARE:
                nc.vector.tensor_scalar(dst, src, 2.0, None, ALU.pow)
            else:
                nc.vector.tensor_tensor(dst, src, src, ALU.mult)

        jets_off = 0
        ev_off = 0
        pending_tail = [None]
        for gi, (g, cap) in enumerate(groups):
            assert cap % 8 == 0 and cap <= 512
            JCg = g * cap
            prs = pairs_of(g)
            PG = len(prs)

            jt = jin.tile([FJ, JCg], bf16, tag="jt")
            nc.sync.dma_start(jt[:], jets_d.ap()[:, jets_off : jets_off + JCg])

            # ---- jets side: 4 layers, layer-major 1024-col chunks.
            def layer(dst, wt, src, width, evac):
                for c0 in range(0, width, 1024):
                    w = min(1024, width - c0)
                    ps = mm.tile([H, 1024], f32, tag="mm")
                    for s0 in range(0, w, 512):
                        sw = min(512, w - s0)
                        nc.tensor.matmul(ps[:, s0 : s0 + sw], wt[:],
                                         src[:, c0 + s0 : c0 + s0 + sw],
                                         start=True, stop=True)
                    evac(dst[:, c0 : c0 + w], ps[:, :w])

            def act_relu(bias_col):
                def f(dst, ps):
                    nc.scalar.activation(dst, ps, AF.Relu,
                                         bias=bv[:, bias_col : bias_col + 1])
                return f

            def act_copy(dst, ps):
                nc.scalar.copy(dst, ps)

            x1 = x12.tile([H, JCg], bf16, tag="x1")
            layer(x1, w1t, jt, JCg, act_relu(0))
            x2 = x12.tile([H, JCg], bf16, tag="x2")
            layer(x2, w2t, x1, JCg, act_relu(1))
            x = bigx.tile([H, JCg], bf16, tag="x")
            layer(x, w3t, x2, JCg, act_relu(2))
            z = bigx.tile([H, JCg], bf16, tag="z")
            layer(z, wzt, x, JCg, act_copy)

            # Tail of the previous group (its last-chunk sums, max tree,
            # accumulator evacs and output DMA) is emitted here so its PE /
            # DVE / Scalar work overlaps this group's jets layers.
            if pending_tail[0] is not None:
                pending_tail[0]()
                pending_tail[0] = None

            xsq = bigx.tile([H, JCg], bf16, tag="xsq")
            square(xsq[:], x[:])

            agg6 = aggs.tile([H, 6 * cap], f32, tag="agg6")

            # ---- x-side trees on DVE (sum exact-halving, max overlap-halving)
            def sum_tree(src_tile, nslices, dst_f32):
                m, cur, off = nslices, src_tile, 0
                if m == 1:
                    nc.vector.tensor_copy(dst_f32, cur[:, 0:cap])
                    return
                while m > 1:
                    k2 = m // 2
                    if k2 == 1:
                        nxt = dst_f32
                    else:
                        nxt = mxp.tile([H, k2 * cap], bf16, tag="xt")
                    nc.vector.tensor_tensor(
                        r3(nxt[:, 0 : k2 * cap], k2),
                        r3(cur[:, off : off + k2 * cap], k2),
                        r3(cur[:, off + k2 * cap : off + 2 * k2 * cap], k2),
                        ALU.add)
                    if m % 2:
                        nc.vector.tensor_add(
                            nxt[:, 0:cap], nxt[:, 0:cap],
                            cur[:, off + (m - 1) * cap : off + m * cap])
                    cur, off, m = nxt, 0, k2

            def max_tree(src_tile, nslices, dst_f32, tag, l1_pool=False,
                         cap=cap):
                m, cur, off = nslices, src_tile, 0
                if m == 1:
                    nc.vector.tensor_copy(dst_f32, src_tile[:, 0:cap])
                    return
                first = True
                while m > 1:
                    k2 = (m + 1) // 2
                    if k2 == 1:
                        nxt = dst_f32
                    else:
                        nxt = mxp.tile([H, k2 * cap], bf16, tag=tag)
                    a0 = cur[:, off : off + k2 * cap]
                    a1 = cur[:, off + (m - k2) * cap : off + m * cap]
                    nc.vector.tensor_tensor(r3(nxt[:, 0 : k2 * cap], k2),
                                            r3(a0, k2), r3(a1, k2), ALU.max)
                    cur, off, m = nxt, 0, k2
                    first = False

            sum_tree(x, g, agg6[:, 0:cap])
            sum_tree(xsq, g, agg6[:, cap : 2 * cap])
            max_tree(x, g, agg6[:, 2 * cap : 3 * cap], "xt")

            # ---- pairs side: SC-slice chunks, lag-1 ysq+sums.
            SC = max(1, 2048 // cap)
            PCH = SC * cap
            y3 = bigy.tile([H, PG * cap], bf16, tag="y3")
            a_sy = acc.tile([H, cap], f32, tag="a_sy")
            a_qy = acc.tile([H, cap], f32, tag="a_qy")

            def ysq_sums(p0, k, cap=cap, PCH=PCH, y3=y3, PG=PG,
                         a_sy=a_sy, a_qy=a_qy):
                w = k * cap
                ysq = scr.tile([H, PCH], bf16, tag="ysq")
                square(ysq[:, :w], y3[:, p0 * cap : p0 * cap + w])
                for s in range(k):
                    sl = p0 + s
                    nc.tensor.matmul(a_sy[:, :cap], ip_t[:],
                                     y3[:, sl * cap : (sl + 1) * cap],
                                     start=(sl == 0), stop=(sl == PG - 1))
                for s in range(k):
                    sl = p0 + s
                    nc.tensor.matmul(a_qy[:, :cap], ip_t[:],
                                     ysq[:, s * cap : (s + 1) * cap],
                                     start=(sl == 0), stop=(sl == PG - 1))

            prev = None
            y3_i = [0]
            for p0 in range(0, PG, SC):
                k = min(SC, PG - p0)
                w = k * cap
                y1 = scr.tile([H, PCH], bf16, tag="y1")
                s = 0
                while s < k:
                    i = prs[p0 + s][0]
                    r = 1
                    while s + r < k and prs[p0 + s + r][0] == i:
                        r += 1
                    j0 = prs[p0 + s][1]
                    nc.vector.tensor_tensor(
                        r3(y1[:, s * cap : (s + r) * cap], r),
                        r3(z[:, i * cap : (i + 1) * cap], 1).broadcast_to(
                            [H, r, cap]),
                        r3(z[:, j0 * cap : (j0 + r) * cap], r),
                        ALU.add)
                    s += r
                nc.vector.tensor_scalar(y1[:, :w], y1[:, :w], bv[:, 3:4],
                                        0.0, ALU.add, ALU.max)
                y2 = scr.tile([H, PCH], bf16, tag="y2")
                for n0 in range(0, w, 1024):
                    cw = min(1024, w - n0)
                    ps = mm.tile([H, 1024], f32, tag="mm")
                    for s0 in range(0, cw, 512):
                        sw = min(512, cw - s0)
                        nc.tensor.matmul(ps[:, s0 : s0 + sw], w4t[:],
                                         y1[:, n0 + s0 : n0 + s0 + sw],
                                         start=True, stop=True)
                    nc.scalar.activation(y2[:, n0 : n0 + cw], ps[:, :cw],
                                         AF.Relu, bias=bv[:, 4:5])
                for n0 in range(0, w, 1024):
                    cw = min(1024, w - n0)
                    ps = mm.tile([H, 1024], f32, tag="mm")
                    for s0 in range(0, cw, 512):
                        sw = min(512, cw - s0)
                        nc.tensor.matmul(ps[:, s0 : s0 + sw], w5t[:],
                                         y2[:, n0 + s0 : n0 + s0 + sw],
                                         start=True, stop=True)
                    dst = y3[:, p0 * cap + n0 : p0 * cap + n0 + cw]
                    if y3_i[0] % 2 == 0:
                        nc.scalar.activation(dst, ps[:, :cw], AF.Relu,
                                             bias=bv[:, 5:6])
                    else:
                        nc.vector.tensor_scalar(dst, ps[:, :cw], bv[:, 5:6],
                                                0.0, ALU.add, ALU.max)
                    y3_i[0] += 1
                if prev is not None:
                    ysq_sums(*prev)
                prev = (p0, k)

            def tail(prev=prev, y3=y3, PG=PG, cap=cap, agg6=agg6,
                     a_sy=a_sy, a_qy=a_qy, ev_off=ev_off, max_tree=max_tree,
                     ysq_sums=ysq_sums):
                ysq_sums(*prev)
                max_tree(y3, PG, agg6[:, 5 * cap : 6 * cap], "yt",
                         l1_pool=True)
                nc.scalar.copy(agg6[:, 3 * cap : 4 * cap], a_sy[:, :cap])
                nc.scalar.copy(agg6[:, 4 * cap : 5 * cap], a_qy[:, :cap])
                nc.sync.dma_start(
                    out6_d.ap()[:, 6 * ev_off : 6 * ev_off + 6 * cap],
                    agg6[:])

            pending_tail[0] = tail

            jets_off += JCg
            ev_off += cap
        pending_tail[0]()

    nc.compile()
    return nc


# ---------------- host-side math ----------------

BN_EPS = 1e-3


def fold_params(inp):
    """Fold normalization + BN into per-layer (W, b). All numpy fp32."""
    mean_j = np.asarray(inp["mean_jets"], np.float32)
    std_j = np.asarray(inp["std_jets"], np.float32)
    w1f = np.asarray(inp["w1_first"], np.float32)
    w1r = np.asarray(inp["w1_rest"], np.float32)
    bn1 = np.asarray(inp["bn1"], np.float32)  # [3,4,H]: gamma, beta, mean, var
    w2f = np.asarray(inp["w2_first"], np.float32)
    w2r = np.asarray(inp["w2_rest"], np.float32)
    bn2 = np.asarray(inp["bn2"], np.float32)

    def bn_sb(row):
        gm, bt, mu, vv = row[0], row[1], row[2], row[3]
        s = gm / np.sqrt(vv + BN_EPS)
        return s.astype(np.float32), (bt - mu * s).astype(np.float32)

    s11, t11 = bn_sb(bn1[0]); s12, t12 = bn_sb(bn1[1]); s13, t13 = bn_sb(bn1[2])
    s21, t21 = bn_sb(bn2[0]); s22, t22 = bn_sb(bn2[1]); s23, t23 = bn_sb(bn2[2])

    A = w1f / std_j[:, None]
    c = -(mean_j / std_j) @ w1f
    return dict(
        W1=A * s11[None, :], b1=c * s11 + t11,
        W2=w1r[0] * s12[None, :], b2=t12,
        W3=w1r[1] * s13[None, :], b3=t13,
        Wz=w2f * s21[None, :], bz=t21,
        W4=w2r[0] * s22[None, :], b4=t22,
        W5=w2r[1] * s23[None, :], b5=t23,
    )


# ---------------- full kernel entry point ----------------

N_CORES = 8

_cache = {}
_TRACE = [False]
_LAST_RESULT = [None]


def _get_program(groups_key):
    if groups_key not in _cache:
        _cache[groups_key] = build_program(list(groups_key))
    return _cache[groups_key]


def _np_dt(dt):
    return mybir.dt.np(dt)


def _plan(n):
    """Returns (groups, slots): groups = [(g, cap)], slots[c][gi] =
    (padded index array, real count) for core c, group gi."""
    gs = []
    idx_by_g = {}
    for g in range(2, 11):
        idx = np.nonzero(n == g)[0]
        if len(idx):
            gs.append(g)
            idx_by_g[g] = idx
    stray = np.nonzero((n < 2) | (n > 10))[0]
    if len(stray):
        if not gs:
            gs.append(2)
            idx_by_g[2] = stray
        else:
            idx_by_g[gs[-1]] = np.concatenate([idx_by_g[gs[-1]], stray])
    groups = []
    slots = [[] for _ in range(N_CORES)]
    for g in gs:
        idx = idx_by_g[g]
        per_core = [idx[c::N_CORES] for c in range(N_CORES)]
        mx = max(len(p) for p in per_core)
        cap = max(8, ((mx + 7) // 8) * 8)
        groups.append((g, cap))
        fill = idx[0]
        for c in range(N_CORES):
            p = per_core[c]
            pad = np.full(cap, p[0] if len(p) else fill, dtype=np.int64)
            pad[: len(p)] = p
            slots[c].append((pad, len(p)))
    return groups, slots


def _pack_jets(jets, groups, slots_c):
    cols = []
    for (g, cap), (ids, _cnt) in zip(groups, slots_c):
        ev = jets[ids][:, :g, :]  # [cap, g, 16]
        cols.append(np.ascontiguousarray(ev.transpose(2, 1, 0)).reshape(
            FJ, g * cap))
    return np.concatenate(cols, axis=1).astype(_np_dt(bf16), copy=False)


def kernel(**inputs):
    from concourse.bass_utils import run_bass_kernel_spmd

    jets = np.asarray(inputs["inputs_jets"], dtype=np.float32)
    B = jets.shape[0]
    mask = (jets != 0.0).any(-1)
    n = mask.sum(-1).astype(np.int64)
    # compact valid jets to the front (no-op for the standard generator)
    if not np.array_equal(mask, np.arange(jets.shape[1])[None, :] < n[:, None]):
        order = np.argsort(~mask, axis=1, kind="stable")
        jets = np.take_along_axis(jets, order[:, :, None], axis=1)

    P = fold_params(inputs)
    groups, slots = _plan(n)
    nc = _get_program(tuple(groups))

    bvec = np.zeros((H, 8), np.float32)
    for i, k in enumerate(["b1", "b2", "b3", "bz", "b4", "b5"]):
        bvec[:, i] = P[k]
    ident = np.eye(H, dtype=np.float32)
    bnp = _np_dt(bf16)
    common = {
        "w1": P["W1"].astype(bnp), "w2": P["W2"].astype(bnp),
        "w3": P["W3"].astype(bnp), "wz": P["Wz"].astype(bnp),
        "w4": P["W4"].astype(bnp), "w5": P["W5"].astype(bnp),
        "identp": ident.astype(bnp), "bvec": bvec,
    }
    in_maps = []
    for c in range(N_CORES):
        m = dict(common)
        m["jets"] = _pack_jets(jets, groups, slots[c])
        in_maps.append(m)

    res = run_bass_kernel_spmd(nc, in_maps, core_ids=list(range(N_CORES)),
                               trace=_TRACE[0])
    _LAST_RESULT[0] = res

    agg_x = np.empty((B, 4 * H), np.float32)
    agg_y = np.empty((B, 4 * H), np.float32)
    for c in range(N_CORES):
        o6 = res.results[c]["out6"]  # [H, 6*EC] f32
        ev_off = 0
        for (g, cap), (ids, cnt) in zip(groups, slots[c]):
            blk = o6[:, 6 * ev_off : 6 * ev_off + 6 * cap]
            sx = blk[:, 0:cap].T[:cnt]
            qx = blk[:, cap : 2 * cap].T[:cnt]
            mx = blk[:, 2 * cap : 3 * cap].T[:cnt]
            sy = blk[:, 3 * cap : 4 * cap].T[:cnt]
            qy = blk[:, 4 * cap : 5 * cap].T[:cnt]
            my = blk[:, 5 * cap : 6 * cap].T[:cnt]
            ii = ids[:cnt]
            mean_x = sx / g
            mean_y = sy / (g * (g - 1) // 2)
            agg_x[ii] = np.concatenate(
                [sx, mx, mean_x, qx / g - mean_x * mean_x], axis=1)
            agg_y[ii] = np.concatenate(
                [sy, my, mean_y, qy / (g * (g - 1) // 2) - mean_y * mean_y],
                axis=1)
            ev_off += cap
    return agg_x, agg_y


# revision 36
# speedup vs baseline: 1.3051x; 1.0293x over previous
# DeepSet Trainium2 kernel.
#
# Strategy: events are sorted by jet-count n (2..10) on the host and
# round-robin sharded across 8 cores into per-group slots of capacity cap_g
# (multiple of 8, exact-packed). Within a group every event has exactly n=g
# valid jets, so all masks, pair structures and aggregation counts are
# compile-time constants.
#
# Math folding (host, O(params)):
#   every Dense+BN+relu block becomes relu(h @ W' + b') with W', b' folded.
#   MLP2 layer 1 uses the z-trick: y1 = relu(z_i + z_j + t) with z = x @ Wz'.
#   t is folded into the y1 relu pass (tensor_scalar add-bias+max0).
#
# Device layout: feature-major [H=128 partitions, columns = slice*cap + b]
# per group, all activations bf16 (PE: 1 col/cycle), PSUM f32.
# Work distribution across engines:
#   PE:   all matmuls + Sum/Sumsq of pairs via PSUM-accumulating identity mms
#   Act:  x1/x2/z/x PSUM evacs (relu+bias / copy), y2 evac, xsq (Square)
#   DVE:  x-side sum/sumsq/max trees, y1 add (broadcast AP) + relu,
#         ysq (y3*y3), y-side max tree
#   Pool: y3 PSUM evac (tensor_scalar bias+relu)
# Mean/Var and the final [events, 4H] transpose are computed on the HOST
# from the 6 DMA'd feature-major aggregates (sum/sumsq/max per side).
import math
from contextlib import ExitStack

import numpy as np

import concourse.bass as bass
import concourse.bacc as bacc
import concourse.tile as tile
import concourse.mybir as mybir

f32 = mybir.dt.float32
bf16 = mybir.dt.bfloat16
AF = mybir.ActivationFunctionType
ALU = mybir.AluOpType

H = 128
FJ = 16


def pairs_of(g):
    return [(i, j) for i in range(g) for j in range(i + 1, g)]


# GPSIMD findings (measured): cannot access PSUM (birverifier), has no
# max opcode, tensor_scalar runs ~12ns/col, and even its decent-rate
# tensor_tensor mult slows the Vector engine ~20-30% via SBUF port
# contention. Net negative everywhere -> unused.
# pow is rejected by the DVE ISA check (tensor_scalar_valid_ops).
USE_POW_SQUARE = False


def build_program(groups):
    """groups: list of (g, cap) with cap a multiple of 8, cap <= 512."""
    JC = sum(g * cap for g, cap in groups)
    EC = sum(cap for _, cap in groups)

    nc = bacc.Bacc("TRN2", target_bir_lowering=False, debug=False)

    jets_d = nc.dram_tensor("jets", [FJ, JC], bf16, kind="ExternalInput")
    w1_d = nc.dram_tensor("w1", [FJ, H], bf16, kind="ExternalInput")
    w2_d = nc.dram_tensor("w2", [H, H], bf16, kind="ExternalInput")
    w3_d = nc.dram_tensor("w3", [H, H], bf16, kind="ExternalInput")
    wz_d = nc.dram_tensor("wz", [H, H], bf16, kind="ExternalInput")
    w4_d = nc.dram_tensor("w4", [H, H], bf16, kind="ExternalInput")
    w5_d = nc.dram_tensor("w5", [H, H], bf16, kind="ExternalInput")
    identp_d = nc.dram_tensor("identp", [H, H], bf16, kind="ExternalInput")
    # bias vector cols: 0..5 = b1, b2, b3, t(=bz), b4, b5
    bv_d = nc.dram_tensor("bvec", [H, 8], f32, kind="ExternalInput")
    # per group: 6 aggregates [H, cap] each, packed [sx qx mx sy qy my]
    out6_d = nc.dram_tensor("out6", [H, 6 * EC], f32, kind="ExternalOutput")

    with tile.TileContext(nc) as tc, ExitStack() as ctx:
        consts = ctx.enter_context(tc.tile_pool(name="consts", bufs=1))
        jin = ctx.enter_context(tc.tile_pool(name="jin", bufs=2))
        x12 = ctx.enter_context(tc.tile_pool(name="x12", bufs=2))
        bigx = ctx.enter_context(tc.tile_pool(name="bigx", bufs=2))
        bigy = ctx.enter_context(tc.tile_pool(name="bigy", bufs=2))
        scr = ctx.enter_context(tc.tile_pool(name="scr", bufs=2))
        mxp = ctx.enter_context(tc.tile_pool(name="mxp", bufs=2))
        aggs = ctx.enter_context(tc.tile_pool(name="aggs", bufs=2))
        mm = ctx.enter_context(tc.tile_pool(name="mm", bufs=2, space="PSUM"))
        acc = ctx.enter_context(tc.tile_pool(name="acc", bufs=2, space="PSUM"))

        def const_tile(name, dram, shape, dt):
            t = consts.tile(shape, dt, tag=name)
            nc.sync.dma_start(t[:], dram.ap())
            return t

        w1t = const_tile("w1", w1_d, [FJ, H], bf16)
        w2t = const_tile("w2", w2_d, [H, H], bf16)
        w3t = const_tile("w3", w3_d, [H, H], bf16)
        wzt = const_tile("wz", wz_d, [H, H], bf16)
        w4t = const_tile("w4", w4_d, [H, H], bf16)
        w5t = const_tile("w5", w5_d, [H, H], bf16)
        ip_t = const_tile("ip", identp_d, [H, H], bf16)
        bv = const_tile("bv", bv_d, [H, 8], f32)

        def r3(ap, k):
            return ap.rearrange("p (k c) -> p k c", k=k)

        # square via DVE tensor_scalar pow: hits the 4x bf16 perf mode
        # (tensor_tensor mult only reaches ~1 elem/cycle). Inputs are
        # relu outputs (>= 0) so pow is safe.
        def square(dst, src):
            if USE_POW_SQUARE:
                nc.vector.tensor_scalar(dst, src, 2.0, None, ALU.pow)
            else:
                nc.vector.tensor_tensor(dst, src, src, ALU.mult)

        jets_off = 0
        ev_off = 0
        pending_tail = [None]
        for gi, (g, cap) in enumerate(groups):
            assert cap % 8 == 0 and cap <= 512
            JCg = g * cap
            prs = pairs_of(g)
            PG = len(prs)

            jt = jin.tile([FJ, JCg], bf16, tag="jt")
            nc.sync.dma_start(jt[:], jets_d.ap()[:, jets_off : jets_off + JCg])

            # ---- jets side: 4 layers, layer-major 1024-col chunks.
            def layer(dst, wt, src, width, evac):
                for c0 in range(0, width, 1024):
                    w = min(1024, width - c0)
                    ps = mm.tile([H, 1024], f32, tag="mm")
                    for s0 in range(0, w, 512):
                        sw = min(512, w - s0)
                        nc.tensor.matmul(ps[:, s0 : s0 + sw], wt[:],
                                         src[:, c0 + s0 : c0 + s0 + sw],
                                         start=True, stop=True)
                    evac(dst[:, c0 : c0 + w], ps[:, :w])

            def act_relu(bias_col):
                def f(dst, ps):
                    nc.scalar.activation(dst, ps, AF.Relu,
                                         bias=bv[:, bias_col : bias_col + 1])
                return f

            def act_copy(dst, ps):
                nc.scalar.copy(dst, ps)

            x1 = x12.tile([H, JCg], bf16, tag="x1")
            layer(x1, w1t, jt, JCg, act_relu(0))
            x2 = x12.tile([H, JCg], bf16, tag="x2")
            layer(x2, w2t, x1, JCg, act_relu(1))
            x = bigx.tile([H, JCg], bf16, tag="x")
            layer(x, w3t, x2, JCg, act_relu(2))
            z = bigx.tile([H, JCg], bf16, tag="z")
            layer(z, wzt, x, JCg, act_copy)

            # Tail of the previous group (its last-chunk sums, max tree,
            # accumulator evacs and output DMA) is emitted here so its PE /
            # DVE / Scalar work overlaps this group's jets layers.
            if pending_tail[0] is not None:
                pending_tail[0]()
                pending_tail[0] = None

            agg6 = aggs.tile([H, 6 * cap], f32, tag="agg6")

            # ---- x-side trees on DVE (sum exact-halving, max overlap-halving)
            def sum_tree(src_tile, nslices, dst_f32):
                m, cur, off = nslices, src_tile, 0
                if m == 1:
                    nc.vector.tensor_copy(dst_f32, cur[:, 0:cap])
                    return
                while m > 1:
                    k2 = m // 2
                    if k2 == 1:
                        nxt = dst_f32
                    else:
                        nxt = mxp.tile([H, k2 * cap], bf16, tag="xt")
                    nc.vector.tensor_tensor(
                        r3(nxt[:, 0 : k2 * cap], k2),
                        r3(cur[:, off : off + k2 * cap], k2),
                        r3(cur[:, off + k2 * cap : off + 2 * k2 * cap], k2),
                        ALU.add)
                    if m % 2:
                        nc.vector.tensor_add(
                            nxt[:, 0:cap], nxt[:, 0:cap],
                            cur[:, off + (m - 1) * cap : off + m * cap])
                    cur, off, m = nxt, 0, k2

            def max_tree(src_tile, nslices, dst_f32, tag, l1_pool=False,
                         cap=cap):
                m, cur, off = nslices, src_tile, 0
                if m == 1:
                    nc.vector.tensor_copy(dst_f32, src_tile[:, 0:cap])
                    return
                first = True
                while m > 1:
                    k2 = (m + 1) // 2
                    if k2 == 1:
                        nxt = dst_f32
                    else:
                        nxt = mxp.tile([H, k2 * cap], bf16, tag=tag)
                    a0 = cur[:, off : off + k2 * cap]
                    a1 = cur[:, off + (m - k2) * cap : off + m * cap]
                    nc.vector.tensor_tensor(r3(nxt[:, 0 : k2 * cap], k2),
                                            r3(a0, k2), r3(a1, k2), ALU.max)
                    cur, off, m = nxt, 0, k2
                    first = False

            # ---- pairs side: SC-slice chunks, lag-1 ysq+sums.
            SC = max(1, 2048 // cap)
            PCH = SC * cap
            y3 = bigy.tile([H, PG * cap], bf16, tag="y3")
            a_sy = acc.tile([H, cap], f32, tag="a_sy")
            a_qy = acc.tile([H, cap], f32, tag="a_qy")

            def ysq_sums(p0, k, cap=cap, PCH=PCH, y3=y3, PG=PG,
                         a_sy=a_sy, a_qy=a_qy):
                w = k * cap
                ysq = scr.tile([H, PCH], bf16, tag="ysq")
                square(ysq[:, :w], y3[:, p0 * cap : p0 * cap + w])
                for s in range(k):
                    sl = p0 + s
                    nc.tensor.matmul(a_sy[:, :cap], ip_t[:],
                                     y3[:, sl * cap : (sl + 1) * cap],
                                     start=(sl == 0), stop=(sl == PG - 1))
                for s in range(k):
                    sl = p0 + s
                    nc.tensor.matmul(a_qy[:, :cap], ip_t[:],
                                     ysq[:, s * cap : (s + 1) * cap],
                                     start=(sl == 0), stop=(sl == PG - 1))

            prev = None
            y3_i = [0]
            for p0 in range(0, PG, SC):
                k = min(SC, PG - p0)
                w = k * cap
                y1 = scr.tile([H, PCH], bf16, tag="y1")
                s = 0
                while s < k:
                    i = prs[p0 + s][0]
                    r = 1
                    while s + r < k and prs[p0 + s + r][0] == i:
                        r += 1
                    j0 = prs[p0 + s][1]
                    nc.vector.tensor_tensor(
                        r3(y1[:, s * cap : (s + r) * cap], r),
                        r3(z[:, i * cap : (i + 1) * cap], 1).broadcast_to(
                            [H, r, cap]),
                        r3(z[:, j0 * cap : (j0 + r) * cap], r),
                        ALU.add)
                    s += r
                nc.vector.tensor_scalar(y1[:, :w], y1[:, :w], bv[:, 3:4],
                                        0.0, ALU.add, ALU.max)
                y2 = scr.tile([H, PCH], bf16, tag="y2")
                for n0 in range(0, w, 1024):
                    cw = min(1024, w - n0)
                    ps = mm.tile([H, 1024], f32, tag="mm")
                    for s0 in range(0, cw, 512):
                        sw = min(512, cw - s0)
                        nc.tensor.matmul(ps[:, s0 : s0 + sw], w4t[:],
                                         y1[:, n0 + s0 : n0 + s0 + sw],
                                         start=True, stop=True)
                    nc.scalar.activation(y2[:, n0 : n0 + cw], ps[:, :cw],
                                         AF.Relu, bias=bv[:, 4:5])
                for n0 in range(0, w, 1024):
                    cw = min(1024, w - n0)
                    ps = mm.tile([H, 1024], f32, tag="mm")
                    for s0 in range(0, cw, 512):
                        sw = min(512, cw - s0)
                        nc.tensor.matmul(ps[:, s0 : s0 + sw], w5t[:],
                                         y2[:, n0 + s0 : n0 + s0 + sw],
                                         start=True, stop=True)
                    dst = y3[:, p0 * cap + n0 : p0 * cap + n0 + cw]
                    if y3_i[0] % 2 == 0:
                        nc.scalar.activation(dst, ps[:, :cw], AF.Relu,
                                             bias=bv[:, 5:6])
                    else:
                        nc.vector.tensor_scalar(dst, ps[:, :cw], bv[:, 5:6],
                                                0.0, ALU.add, ALU.max)
                    y3_i[0] += 1
                if prev is not None:
                    ysq_sums(*prev)
                prev = (p0, k)

            # x-side squares/trees emitted after the pair chunks: the
            # Vector engine runs them while PE finishes the pair matmuls,
            # instead of delaying the first y1 (which PE waits on).
            xsq = bigx.tile([H, JCg], bf16, tag="xsq")
            square(xsq[:], x[:])
            sum_tree(x, g, agg6[:, 0:cap])
            sum_tree(xsq, g, agg6[:, cap : 2 * cap])
            max_tree(x, g, agg6[:, 2 * cap : 3 * cap], "xt")

            def tail(prev=prev, y3=y3, PG=PG, cap=cap, agg6=agg6,
                     a_sy=a_sy, a_qy=a_qy, ev_off=ev_off, max_tree=max_tree,
                     ysq_sums=ysq_sums):
                ysq_sums(*prev)
                max_tree(y3, PG, agg6[:, 5 * cap : 6 * cap], "yt",
                         l1_pool=True)
                nc.scalar.copy(agg6[:, 3 * cap : 4 * cap], a_sy[:, :cap])
                nc.scalar.copy(agg6[:, 4 * cap : 5 * cap], a_qy[:, :cap])
                nc.sync.dma_start(
                    out6_d.ap()[:, 6 * ev_off : 6 * ev_off + 6 * cap],
                    agg6[:])

            pending_tail[0] = tail

            jets_off += JCg
            ev_off += cap
        pending_tail[0]()

    nc.compile()
    return nc


# ---------------- host-side math ----------------

BN_EPS = 1e-3


def fold_params(inp):
    """Fold normalization + BN into per-layer (W, b). All numpy fp32."""
    mean_j = np.asarray(inp["mean_jets"], np.float32)
    std_j = np.asarray(inp["std_jets"], np.float32)
    w1f = np.asarray(inp["w1_first"], np.float32)
    w1r = np.asarray(inp["w1_rest"], np.float32)
    bn1 = np.asarray(inp["bn1"], np.float32)  # [3,4,H]: gamma, beta, mean, var
    w2f = np.asarray(inp["w2_first"], np.float32)
    w2r = np.asarray(inp["w2_rest"], np.float32)
    bn2 = np.asarray(inp["bn2"], np.float32)

    def bn_sb(row):
        gm, bt, mu, vv = row[0], row[1], row[2], row[3]
        s = gm / np.sqrt(vv + BN_EPS)
        return s.astype(np.float32), (bt - mu * s).astype(np.float32)

    s11, t11 = bn_sb(bn1[0]); s12, t12 = bn_sb(bn1[1]); s13, t13 = bn_sb(bn1[2])
    s21, t21 = bn_sb(bn2[0]); s22, t22 = bn_sb(bn2[1]); s23, t23 = bn_sb(bn2[2])

    A = w1f / std_j[:, None]
    c = -(mean_j / std_j) @ w1f
    return dict(
        W1=A * s11[None, :], b1=c * s11 + t11,
        W2=w1r[0] * s12[None, :], b2=t12,
        W3=w1r[1] * s13[None, :], b3=t13,
        Wz=w2f * s21[None, :], bz=t21,
        W4=w2r[0] * s22[None, :], b4=t22,
        W5=w2r[1] * s23[None, :], b5=t23,
    )


# ---------------- full kernel entry point ----------------

N_CORES = 8

_cache = {}
_TRACE = [False]
_LAST_RESULT = [None]


def _get_program(groups_key):
    if groups_key not in _cache:
        _cache[groups_key] = build_program(list(groups_key))
    return _cache[groups_key]


def _np_dt(dt):
    return mybir.dt.np(dt)


def _plan(n):
    """Returns (groups, slots): groups = [(g, cap)], slots[c][gi] =
    (padded index array, real count) for core c, group gi."""
    gs = []
    idx_by_g = {}
    for g in range(2, 11):
        idx = np.nonzero(n == g)[0]
        if len(idx):
            gs.append(g)
            idx_by_g[g] = idx
    stray = np.nonzero((n < 2) | (n > 10))[0]
    if len(stray):
        if not gs:
            gs.append(2)
            idx_by_g[2] = stray
        else:
            idx_by_g[gs[-1]] = np.concatenate([idx_by_g[gs[-1]], stray])
    # Interleave big/small groups so a small group's serial jets chain
    # overlaps a big group's long pair phase: [10, 2, 9, 3, 8, 4, ...]
    desc = sorted(gs, key=lambda g: -g)
    inter = []
    lo, hi = 0, len(desc) - 1
    while lo <= hi:
        inter.append(desc[lo]); lo += 1
        if lo <= hi:
            inter.append(desc[hi]); hi -= 1
    gs = inter
    groups = []
    slots = [[] for _ in range(N_CORES)]
    for g in gs:
        idx = idx_by_g[g]
        per_core = [idx[c::N_CORES] for c in range(N_CORES)]
        mx = max(len(p) for p in per_core)
        cap = max(8, ((mx + 7) // 8) * 8)
        groups.append((g, cap))
        fill = idx[0]
        for c in range(N_CORES):
            p = per_core[c]
            pad = np.full(cap, p[0] if len(p) else fill, dtype=np.int64)
            pad[: len(p)] = p
            slots[c].append((pad, len(p)))
    return groups, slots


def _pack_jets(jets, groups, slots_c):
    cols = []
    for (g, cap), (ids, _cnt) in zip(groups, slots_c):
        ev = jets[ids][:, :g, :]  # [cap, g, 16]
        cols.append(np.ascontiguousarray(ev.transpose(2, 1, 0)).reshape(
            FJ, g * cap))
    return np.concatenate(cols, axis=1).astype(_np_dt(bf16), copy=False)


def kernel(**inputs):
    from concourse.bass_utils import run_bass_kernel_spmd

    jets = np.asarray(inputs["inputs_jets"], dtype=np.float32)
    B = jets.shape[0]
    mask = (jets != 0.0).any(-1)
    n = mask.sum(-1).astype(np.int64)
    # compact valid jets to the front (no-op for the standard generator)
    if not np.array_equal(mask, np.arange(jets.shape[1])[None, :] < n[:, None]):
        order = np.argsort(~mask, axis=1, kind="stable")
        jets = np.take_along_axis(jets, order[:, :, None], axis=1)

    P = fold_params(inputs)
    groups, slots = _plan(n)
    nc = _get_program(tuple(groups))

    bvec = np.zeros((H, 8), np.float32)
    for i, k in enumerate(["b1", "b2", "b3", "bz", "b4", "b5"]):
        bvec[:, i] = P[k]
    ident = np.eye(H, dtype=np.float32)
    bnp = _np_dt(bf16)
    common = {
        "w1": P["W1"].astype(bnp), "w2": P["W2"].astype(bnp),
        "w3": P["W3"].astype(bnp), "wz": P["Wz"].astype(bnp),
        "w4": P["W4"].astype(bnp), "w5": P["W5"].astype(bnp),
        "identp": ident.astype(bnp), "bvec": bvec,
    }
    in_maps = []
    for c in range(N_CORES):
        m = dict(common)
        m["jets"] = _pack_jets(jets, groups, slots[c])
        in_maps.append(m)

    res = run_bass_kernel_spmd(nc, in_maps, core_ids=list(range(N_CORES)),
                               trace=_TRACE[0])
    _LAST_RESULT[0] = res

    agg_x = np.empty((B, 4 * H), np.float32)
    agg_y = np.empty((B, 4 * H), np.float32)
    for c in range(N_CORES):
        o6 = res.results[c]["out6"]  # [H, 6*EC] f32
        ev_off = 0
        for (g, cap), (ids, cnt) in zip(groups, slots[c]):
            blk = o6[:, 6 * ev_off : 6 * ev_off + 6 * cap]
            sx = blk[:, 0:cap].T[:cnt]
            qx = blk[:, cap : 2 * cap].T[:cnt]
            mx = blk[:, 2 * cap : 3 * cap].T[:cnt]
            sy = blk[:, 3 * cap : 4 * cap].T[:cnt]
            qy = blk[:, 4 * cap : 5 * cap].T[:cnt]
            my = blk[:, 5 * cap : 6 * cap].T[:cnt]
            ii = ids[:cnt]
            mean_x = sx / g
            mean_y = sy / (g * (g - 1) // 2)
            agg_x[ii] = np.concatenate(
                [sx, mx, mean_x, qx / g - mean_x * mean_x], axis=1)
            agg_y[ii] = np.concatenate(
                [sy, my, mean_y, qy / (g * (g - 1) // 2) - mean_y * mean_y],
                axis=1)
            ev_off += cap
    return agg_x, agg_y


# revision 40
# speedup vs baseline: 1.3436x; 1.0295x over previous
# DeepSet Trainium2 kernel.
#
# Strategy: events are sorted by jet-count n (2..10) on the host and
# round-robin sharded across 8 cores into per-group slots of capacity cap_g
# (multiple of 8, exact-packed). Within a group every event has exactly n=g
# valid jets, so all masks, pair structures and aggregation counts are
# compile-time constants.
#
# Math folding (host, O(params)):
#   every Dense+BN+relu block becomes relu(h @ W' + b') with W', b' folded.
#   MLP2 layer 1 uses the z-trick: y1 = relu(z_i + z_j + t) with z = x @ Wz'.
#   t is folded into the y1 relu pass (tensor_scalar add-bias+max0).
#
# Device layout: feature-major [H=128 partitions, columns = slice*cap + b]
# per group, all activations bf16 (PE: 1 col/cycle), PSUM f32.
# Work distribution across engines:
#   PE:   all matmuls + Sum/Sumsq of pairs via PSUM-accumulating identity mms
#   Act:  x1/x2/z/x PSUM evacs (relu+bias / copy), y2 evac, xsq (Square)
#   DVE:  x-side sum/sumsq/max trees, y1 add (broadcast AP) + relu,
#         ysq (y3*y3), y-side max tree
#   Pool: y3 PSUM evac (tensor_scalar bias+relu)
# Mean/Var and the final [events, 4H] transpose are computed on the HOST
# from the 6 DMA'd feature-major aggregates (sum/sumsq/max per side).
import math
from contextlib import ExitStack

import numpy as np

import concourse.bass as bass
import concourse.bacc as bacc
import concourse.tile as tile
import concourse.mybir as mybir

f32 = mybir.dt.float32
bf16 = mybir.dt.bfloat16
AF = mybir.ActivationFunctionType
ALU = mybir.AluOpType

H = 128
FJ = 16


def pairs_of(g):
    return [(i, j) for i in range(g) for j in range(i + 1, g)]


# GPSIMD findings (measured): cannot access PSUM (birverifier), has no
# max opcode, tensor_scalar runs ~12ns/col, and even its decent-rate
# tensor_tensor mult slows the Vector engine ~20-30% via SBUF port
# contention. Net negative everywhere -> unused.
# pow is rejected by the DVE ISA check (tensor_scalar_valid_ops).
USE_POW_SQUARE = False


def build_program(groups):
    """groups: list of (g, cap) with cap a multiple of 8, cap <= 512."""
    JC = sum(g * cap for g, cap in groups)
    EC = sum(cap for _, cap in groups)

    nc = bacc.Bacc("TRN2", target_bir_lowering=False, debug=False)

    jets_d = nc.dram_tensor("jets", [FJ, JC], bf16, kind="ExternalInput")
    w1_d = nc.dram_tensor("w1", [FJ, H], bf16, kind="ExternalInput")
    w2_d = nc.dram_tensor("w2", [H, H], bf16, kind="ExternalInput")
    w3_d = nc.dram_tensor("w3", [H, H], bf16, kind="ExternalInput")
    wz_d = nc.dram_tensor("wz", [H, H], bf16, kind="ExternalInput")
    w4_d = nc.dram_tensor("w4", [H, H], bf16, kind="ExternalInput")
    w5_d = nc.dram_tensor("w5", [H, H], bf16, kind="ExternalInput")
    identp_d = nc.dram_tensor("identp", [H, H], bf16, kind="ExternalInput")
    # bias vector cols: 0..5 = b1, b2, b3, t(=bz), b4, b5
    bv_d = nc.dram_tensor("bvec", [H, 8], f32, kind="ExternalInput")
    # per group: 6 aggregates [H, cap] each, packed [sx qx mx sy qy my]
    out6_d = nc.dram_tensor("out6", [H, 6 * EC], f32, kind="ExternalOutput")

    with tile.TileContext(nc) as tc, ExitStack() as ctx:
        consts = ctx.enter_context(tc.tile_pool(name="consts", bufs=1))
        jin = ctx.enter_context(tc.tile_pool(name="jin", bufs=2))
        x12 = ctx.enter_context(tc.tile_pool(name="x12", bufs=2))
        bigx = ctx.enter_context(tc.tile_pool(name="bigx", bufs=2))
        bigy = ctx.enter_context(tc.tile_pool(name="bigy", bufs=2))
        scr = ctx.enter_context(tc.tile_pool(name="scr", bufs=2))
        mxp = ctx.enter_context(tc.tile_pool(name="mxp", bufs=2))
        aggs = ctx.enter_context(tc.tile_pool(name="aggs", bufs=2))
        mm = ctx.enter_context(tc.tile_pool(name="mm", bufs=2, space="PSUM"))
        acc = ctx.enter_context(tc.tile_pool(name="acc", bufs=1, space="PSUM"))

        def const_tile(name, dram, shape, dt):
            t = consts.tile(shape, dt, tag=name)
            nc.sync.dma_start(t[:], dram.ap())
            return t

        w1t = const_tile("w1", w1_d, [FJ, H], bf16)
        w2t = const_tile("w2", w2_d, [H, H], bf16)
        w3t = const_tile("w3", w3_d, [H, H], bf16)
        wzt = const_tile("wz", wz_d, [H, H], bf16)
        w4t = const_tile("w4", w4_d, [H, H], bf16)
        w5t = const_tile("w5", w5_d, [H, H], bf16)
        ip_t = const_tile("ip", identp_d, [H, H], bf16)
        bv = const_tile("bv", bv_d, [H, 8], f32)

        def r3(ap, k):
            return ap.rearrange("p (k c) -> p k c", k=k)

        # square via DVE tensor_scalar pow: hits the 4x bf16 perf mode
        # (tensor_tensor mult only reaches ~1 elem/cycle). Inputs are
        # relu outputs (>= 0) so pow is safe.
        def square(dst, src):
            if USE_POW_SQUARE:
                nc.vector.tensor_scalar(dst, src, 2.0, None, ALU.pow)
            else:
                nc.vector.tensor_tensor(dst, src, src, ALU.mult)

        jets_off = 0
        ev_off = 0
        pending_tail = [None]
        for gi, (g, cap) in enumerate(groups):
            assert cap % 8 == 0 and cap <= 512
            JCg = g * cap
            prs = pairs_of(g)
            PG = len(prs)

            jt = jin.tile([FJ, JCg], bf16, tag="jt")
            nc.sync.dma_start(jt[:], jets_d.ap()[:, jets_off : jets_off + JCg])

            # ---- jets side: 4 layers, layer-major 1024-col chunks.
            def layer(dst, wt, src, width, evac):
                for c0 in range(0, width, 1024):
                    w = min(1024, width - c0)
                    ps = mm.tile([H, 1024], f32, tag="mm")
                    for s0 in range(0, w, 512):
                        sw = min(512, w - s0)
                        nc.tensor.matmul(ps[:, s0 : s0 + sw], wt[:],
                                         src[:, c0 + s0 : c0 + s0 + sw],
                                         start=True, stop=True)
                    evac(dst[:, c0 : c0 + w], ps[:, :w])

            def act_relu(bias_col):
                def f(dst, ps):
                    nc.scalar.activation(dst, ps, AF.Relu,
                                         bias=bv[:, bias_col : bias_col + 1])
                return f

            def act_copy(dst, ps):
                nc.scalar.copy(dst, ps)

            x1 = x12.tile([H, JCg], bf16, tag="x1")
            layer(x1, w1t, jt, JCg, act_relu(0))
            x2 = x12.tile([H, JCg], bf16, tag="x2")
            layer(x2, w2t, x1, JCg, act_relu(1))
            x = bigx.tile([H, JCg], bf16, tag="x")
            layer(x, w3t, x2, JCg, act_relu(2))
            z = bigx.tile([H, JCg], bf16, tag="z")
            layer(z, wzt, x, JCg, act_copy)

            # Tail of the previous group (its last-chunk sums, max tree,
            # accumulator evacs and output DMA) is emitted here so its PE /
            # DVE / Scalar work overlaps this group's jets layers.
            if pending_tail[0] is not None:
                pending_tail[0]()
                pending_tail[0] = None

            agg6 = aggs.tile([H, 6 * cap], f32, tag="agg6")

            def max_tree(src_tile, nslices, dst_f32, tag, l1_pool=False,
                         cap=cap):
                m, cur, off = nslices, src_tile, 0
                if m == 1:
                    nc.vector.tensor_copy(dst_f32, src_tile[:, 0:cap])
                    return
                first = True
                while m > 1:
                    k2 = (m + 1) // 2
                    if k2 == 1:
                        nxt = dst_f32
                    else:
                        nxt = mxp.tile([H, k2 * cap], bf16, tag=tag)
                    a0 = cur[:, off : off + k2 * cap]
                    a1 = cur[:, off + (m - k2) * cap : off + m * cap]
                    nc.vector.tensor_tensor(r3(nxt[:, 0 : k2 * cap], k2),
                                            r3(a0, k2), r3(a1, k2), ALU.max)
                    cur, off, m = nxt, 0, k2
                    first = False

            # ---- pairs side: SC-slice chunks, lag-1 ysq+sums.
            SC = max(1, 2048 // cap)
            PCH = SC * cap
            y3 = bigy.tile([H, PG * cap], bf16, tag="y3")
            a_sy = acc.tile([H, cap], f32, tag="a_sy")
            a_qy = acc.tile([H, cap], f32, tag="a_qy")
            a_sx = acc.tile([H, cap], f32, tag="a_sx")
            a_qx = acc.tile([H, cap], f32, tag="a_qx")

            # x-side Sum via PE identity accumulation (f32-exact, frees DVE)
            for s in range(g):
                nc.tensor.matmul(a_sx[:, :cap], ip_t[:],
                                 x[:, s * cap : (s + 1) * cap],
                                 start=(s == 0), stop=(s == g - 1))

            def ysq_sums(p0, k, cap=cap, PCH=PCH, y3=y3, PG=PG,
                         a_sy=a_sy, a_qy=a_qy):
                w = k * cap
                ysq = scr.tile([H, PCH], bf16, tag="ysq")
                square(ysq[:, :w], y3[:, p0 * cap : p0 * cap + w])
                for s in range(k):
                    sl = p0 + s
                    nc.tensor.matmul(a_sy[:, :cap], ip_t[:],
                                     y3[:, sl * cap : (sl + 1) * cap],
                                     start=(sl == 0), stop=(sl == PG - 1))
                for s in range(k):
                    sl = p0 + s
                    nc.tensor.matmul(a_qy[:, :cap], ip_t[:],
                                     ysq[:, s * cap : (s + 1) * cap],
                                     start=(sl == 0), stop=(sl == PG - 1))

            prev = None
            y3_i = [0]
            for p0 in range(0, PG, SC):
                k = min(SC, PG - p0)
                w = k * cap
                y1 = scr.tile([H, PCH], bf16, tag="y1")
                s = 0
                while s < k:
                    i = prs[p0 + s][0]
                    r = 1
                    while s + r < k and prs[p0 + s + r][0] == i:
                        r += 1
                    j0 = prs[p0 + s][1]
                    nc.vector.tensor_tensor(
                        r3(y1[:, s * cap : (s + r) * cap], r),
                        r3(z[:, i * cap : (i + 1) * cap], 1).broadcast_to(
                            [H, r, cap]),
                        r3(z[:, j0 * cap : (j0 + r) * cap], r),
                        ALU.add)
                    s += r
                nc.vector.tensor_scalar(y1[:, :w], y1[:, :w], bv[:, 3:4],
                                        0.0, ALU.add, ALU.max)
                y2 = scr.tile([H, PCH], bf16, tag="y2")
                for n0 in range(0, w, 1024):
                    cw = min(1024, w - n0)
                    ps = mm.tile([H, 1024], f32, tag="mm")
                    for s0 in range(0, cw, 512):
                        sw = min(512, cw - s0)
                        nc.tensor.matmul(ps[:, s0 : s0 + sw], w4t[:],
                                         y1[:, n0 + s0 : n0 + s0 + sw],
                                         start=True, stop=True)
                    nc.scalar.activation(y2[:, n0 : n0 + cw], ps[:, :cw],
                                         AF.Relu, bias=bv[:, 4:5])
                for n0 in range(0, w, 1024):
                    cw = min(1024, w - n0)
                    ps = mm.tile([H, 1024], f32, tag="mm")
                    for s0 in range(0, cw, 512):
                        sw = min(512, cw - s0)
                        nc.tensor.matmul(ps[:, s0 : s0 + sw], w5t[:],
                                         y2[:, n0 + s0 : n0 + s0 + sw],
                                         start=True, stop=True)
                    dst = y3[:, p0 * cap + n0 : p0 * cap + n0 + cw]
                    if y3_i[0] % 2 == 0:
                        nc.scalar.activation(dst, ps[:, :cw], AF.Relu,
                                             bias=bv[:, 5:6])
                    else:
                        nc.vector.tensor_scalar(dst, ps[:, :cw], bv[:, 5:6],
                                                0.0, ALU.add, ALU.max)
                    y3_i[0] += 1
                if prev is not None:
                    ysq_sums(*prev)
                prev = (p0, k)

            # x-side square/max emitted after the pair chunks: the Vector
            # engine runs them while PE finishes the pair matmuls, instead
            # of delaying the first y1 (which PE waits on).
            xsq = bigx.tile([H, JCg], bf16, tag="xsq")
            square(xsq[:], x[:])
            max_tree(x, g, agg6[:, 2 * cap : 3 * cap], "xt")

            def tail(prev=prev, y3=y3, PG=PG, g=g, cap=cap, agg6=agg6,
                     a_sy=a_sy, a_qy=a_qy, a_sx=a_sx, a_qx=a_qx, xsq=xsq,
                     ev_off=ev_off, max_tree=max_tree, ysq_sums=ysq_sums):
                ysq_sums(*prev)
                for s in range(g):
                    nc.tensor.matmul(a_qx[:, :cap], ip_t[:],
                                     xsq[:, s * cap : (s + 1) * cap],
                                     start=(s == 0), stop=(s == g - 1))
                max_tree(y3, PG, agg6[:, 5 * cap : 6 * cap], "yt",
                         l1_pool=True)
                nc.scalar.copy(agg6[:, 0:cap], a_sx[:, :cap])
                nc.scalar.copy(agg6[:, cap : 2 * cap], a_qx[:, :cap])
                nc.scalar.copy(agg6[:, 3 * cap : 4 * cap], a_sy[:, :cap])
                nc.scalar.copy(agg6[:, 4 * cap : 5 * cap], a_qy[:, :cap])
                nc.sync.dma_start(
                    out6_d.ap()[:, 6 * ev_off : 6 * ev_off + 6 * cap],
                    agg6[:])

            pending_tail[0] = tail

            jets_off += JCg
            ev_off += cap
        pending_tail[0]()

    nc.compile()
    return nc


# ---------------- host-side math ----------------

BN_EPS = 1e-3


def fold_params(inp):
    """Fold normalization + BN into per-layer (W, b). All numpy fp32."""
    mean_j = np.asarray(inp["mean_jets"], np.float32)
    std_j = np.asarray(inp["std_jets"], np.float32)
    w1f = np.asarray(inp["w1_first"], np.float32)
    w1r = np.asarray(inp["w1_rest"], np.float32)
    bn1 = np.asarray(inp["bn1"], np.float32)  # [3,4,H]: gamma, beta, mean, var
    w2f = np.asarray(inp["w2_first"], np.float32)
    w2r = np.asarray(inp["w2_rest"], np.float32)
    bn2 = np.asarray(inp["bn2"], np.float32)

    def bn_sb(row):
        gm, bt, mu, vv = row[0], row[1], row[2], row[3]
        s = gm / np.sqrt(vv + BN_EPS)
        return s.astype(np.float32), (bt - mu * s).astype(np.float32)

    s11, t11 = bn_sb(bn1[0]); s12, t12 = bn_sb(bn1[1]); s13, t13 = bn_sb(bn1[2])
    s21, t21 = bn_sb(bn2[0]); s22, t22 = bn_sb(bn2[1]); s23, t23 = bn_sb(bn2[2])

    A = w1f / std_j[:, None]
    c = -(mean_j / std_j) @ w1f
    return dict(
        W1=A * s11[None, :], b1=c * s11 + t11,
        W2=w1r[0] * s12[None, :], b2=t12,
        W3=w1r[1] * s13[None, :], b3=t13,
        Wz=w2f * s21[None, :], bz=t21,
        W4=w2r[0] * s22[None, :], b4=t22,
        W5=w2r[1] * s23[None, :], b5=t23,
    )


# ---------------- full kernel entry point ----------------

N_CORES = 8

_cache = {}
_TRACE = [False]
_LAST_RESULT = [None]


def _get_program(groups_key):
    if groups_key not in _cache:
        _cache[groups_key] = build_program(list(groups_key))
    return _cache[groups_key]


def _np_dt(dt):
    return mybir.dt.np(dt)


def _plan(n):
    """Returns (groups, slots): groups = [(g, cap)], slots[c][gi] =
    (padded index array, real count) for core c, group gi."""
    gs = []
    idx_by_g = {}
    for g in range(2, 11):
        idx = np.nonzero(n == g)[0]
        if len(idx):
            gs.append(g)
            idx_by_g[g] = idx
    stray = np.nonzero((n < 2) | (n > 10))[0]
    if len(stray):
        if not gs:
            gs.append(2)
            idx_by_g[2] = stray
        else:
            idx_by_g[gs[-1]] = np.concatenate([idx_by_g[gs[-1]], stray])
    # Interleave big/small groups so a small group's serial jets chain
    # overlaps a big group's long pair phase: [10, 2, 9, 3, 8, 4, ...]
    desc = sorted(gs, key=lambda g: -g)
    inter = []
    lo, hi = 0, len(desc) - 1
    while lo <= hi:
        inter.append(desc[lo]); lo += 1
        if lo <= hi:
            inter.append(desc[hi]); hi -= 1
    gs = inter
    groups = []
    slots = [[] for _ in range(N_CORES)]
    for g in gs:
        idx = idx_by_g[g]
        per_core = [idx[c::N_CORES] for c in range(N_CORES)]
        mx = max(len(p) for p in per_core)
        cap = max(8, ((mx + 7) // 8) * 8)
        groups.append((g, cap))
        fill = idx[0]
        for c in range(N_CORES):
            p = per_core[c]
            pad = np.full(cap, p[0] if len(p) else fill, dtype=np.int64)
            pad[: len(p)] = p
            slots[c].append((pad, len(p)))
    return groups, slots


def _pack_jets(jets, groups, slots_c):
    cols = []
    for (g, cap), (ids, _cnt) in zip(groups, slots_c):
        ev = jets[ids][:, :g, :]  # [cap, g, 16]
        cols.append(np.ascontiguousarray(ev.transpose(2, 1, 0)).reshape(
            FJ, g * cap))
    return np.concatenate(cols, axis=1).astype(_np_dt(bf16), copy=False)


def kernel(**inputs):
    from concourse.bass_utils import run_bass_kernel_spmd

    jets = np.asarray(inputs["inputs_jets"], dtype=np.float32)
    B = jets.shape[0]
    mask = (jets != 0.0).any(-1)
    n = mask.sum(-1).astype(np.int64)
    # compact valid jets to the front (no-op for the standard generator)
    if not np.array_equal(mask, np.arange(jets.shape[1])[None, :] < n[:, None]):
        order = np.argsort(~mask, axis=1, kind="stable")
        jets = np.take_along_axis(jets, order[:, :, None], axis=1)

    P = fold_params(inputs)
    groups, slots = _plan(n)
    nc = _get_program(tuple(groups))

    bvec = np.zeros((H, 8), np.float32)
    for i, k in enumerate(["b1", "b2", "b3", "bz", "b4", "b5"]):
        bvec[:, i] = P[k]
    ident = np.eye(H, dtype=np.float32)
    bnp = _np_dt(bf16)
    common = {
        "w1": P["W1"].astype(bnp), "w2": P["W2"].astype(bnp),
        "w3": P["W3"].astype(bnp), "wz": P["Wz"].astype(bnp),
        "w4": P["W4"].astype(bnp), "w5": P["W5"].astype(bnp),
        "identp": ident.astype(bnp), "bvec": bvec,
    }
    in_maps = []
    for c in range(N_CORES):
        m = dict(common)
        m["jets"] = _pack_jets(jets, groups, slots[c])
        in_maps.append(m)

    res = run_bass_kernel_spmd(nc, in_maps, core_ids=list(range(N_CORES)),
                               trace=_TRACE[0])
    _LAST_RESULT[0] = res

    agg_x = np.empty((B, 4 * H), np.float32)
    agg_y = np.empty((B, 4 * H), np.float32)
    for c in range(N_CORES):
        o6 = res.results[c]["out6"]  # [H, 6*EC] f32
        ev_off = 0
        for (g, cap), (ids, cnt) in zip(groups, slots[c]):
            blk = o6[:, 6 * ev_off : 6 * ev_off + 6 * cap]
            sx = blk[:, 0:cap].T[:cnt]
            qx = blk[:, cap : 2 * cap].T[:cnt]
            mx = blk[:, 2 * cap : 3 * cap].T[:cnt]
            sy = blk[:, 3 * cap : 4 * cap].T[:cnt]
            qy = blk[:, 4 * cap : 5 * cap].T[:cnt]
            my = blk[:, 5 * cap : 6 * cap].T[:cnt]
            ii = ids[:cnt]
            mean_x = sx / g
            mean_y = sy / (g * (g - 1) // 2)
            agg_x[ii] = np.concatenate(
                [sx, mx, mean_x, qx / g - mean_x * mean_x], axis=1)
            agg_y[ii] = np.concatenate(
                [sy, my, mean_y, qy / (g * (g - 1) // 2) - mean_y * mean_y],
                axis=1)
            ev_off += cap
    return agg_x, agg_y


# revision 44
# speedup vs baseline: 1.3562x; 1.0094x over previous
# DeepSet Trainium2 kernel.
#
# Strategy: events are sorted by jet-count n (2..10) on the host and
# round-robin sharded across 8 cores into per-group slots of capacity cap_g
# (multiple of 8, exact-packed). Within a group every event has exactly n=g
# valid jets, so all masks, pair structures and aggregation counts are
# compile-time constants.
#
# Math folding (host, O(params)):
#   every Dense+BN+relu block becomes relu(h @ W' + b') with W', b' folded.
#   MLP2 layer 1 uses the z-trick: y1 = relu(z_i + z_j + t) with z = x @ Wz'.
#   t is folded into the y1 relu pass (tensor_scalar add-bias+max0).
#
# Device layout: feature-major [H=128 partitions, columns = slice*cap + b]
# per group, all activations bf16 (PE: 1 col/cycle), PSUM f32.
# Work distribution across engines:
#   PE:   all matmuls + Sum/Sumsq of pairs via PSUM-accumulating identity mms
#   Act:  x1/x2/z/x PSUM evacs (relu+bias / copy), y2 evac, xsq (Square)
#   DVE:  x-side sum/sumsq/max trees, y1 add (broadcast AP) + relu,
#         ysq (y3*y3), y-side max tree
#   Pool: y3 PSUM evac (tensor_scalar bias+relu)
# Mean/Var and the final [events, 4H] transpose are computed on the HOST
# from the 6 DMA'd feature-major aggregates (sum/sumsq/max per side).
import math
from contextlib import ExitStack

import numpy as np

import concourse.bass as bass
import concourse.bacc as bacc
import concourse.tile as tile
import concourse.mybir as mybir

f32 = mybir.dt.float32
bf16 = mybir.dt.bfloat16
AF = mybir.ActivationFunctionType
ALU = mybir.AluOpType

H = 128
FJ = 16


def pairs_of(g):
    return [(i, j) for i in range(g) for j in range(i + 1, g)]


# GPSIMD findings (measured): cannot access PSUM (birverifier), has no
# max opcode, tensor_scalar runs ~12ns/col, and even its decent-rate
# tensor_tensor mult slows the Vector engine ~20-30% via SBUF port
# contention. Net negative everywhere -> unused.
# pow is rejected by the DVE ISA check (tensor_scalar_valid_ops).
USE_POW_SQUARE = False


def build_program(groups):
    """groups: list of (g, cap) with cap a multiple of 8, cap <= 512."""
    JC = sum(g * cap for g, cap in groups)
    EC = sum(cap for _, cap in groups)

    nc = bacc.Bacc("TRN2", target_bir_lowering=False, debug=False)

    jets_d = nc.dram_tensor("jets", [FJ, JC], bf16, kind="ExternalInput")
    w1_d = nc.dram_tensor("w1", [FJ, H], bf16, kind="ExternalInput")
    w2_d = nc.dram_tensor("w2", [H, H], bf16, kind="ExternalInput")
    w3_d = nc.dram_tensor("w3", [H, H], bf16, kind="ExternalInput")
    wz_d = nc.dram_tensor("wz", [H, H], bf16, kind="ExternalInput")
    w4_d = nc.dram_tensor("w4", [H, H], bf16, kind="ExternalInput")
    w5_d = nc.dram_tensor("w5", [H, H], bf16, kind="ExternalInput")
    identp_d = nc.dram_tensor("identp", [H, H], bf16, kind="ExternalInput")
    # bias vector cols: 0..5 = b1, b2, b3, t(=bz), b4, b5
    bv_d = nc.dram_tensor("bvec", [H, 8], f32, kind="ExternalInput")
    # per group: 6 aggregates [H, cap] each, packed [sx qx mx sy qy my]
    out6_d = nc.dram_tensor("out6", [H, 6 * EC], f32, kind="ExternalOutput")

    with tile.TileContext(nc) as tc, ExitStack() as ctx:
        consts = ctx.enter_context(tc.tile_pool(name="consts", bufs=1))
        jin = ctx.enter_context(tc.tile_pool(name="jin", bufs=2))
        x12 = ctx.enter_context(tc.tile_pool(name="x12", bufs=2))
        bigx = ctx.enter_context(tc.tile_pool(name="bigx", bufs=2))
        bigy = ctx.enter_context(tc.tile_pool(name="bigy", bufs=2))
        scr = ctx.enter_context(tc.tile_pool(name="scr", bufs=2))
        mxp = ctx.enter_context(tc.tile_pool(name="mxp", bufs=2))
        aggs = ctx.enter_context(tc.tile_pool(name="aggs", bufs=2))
        mm = ctx.enter_context(tc.tile_pool(name="mm", bufs=2, space="PSUM"))
        acc = ctx.enter_context(tc.tile_pool(name="acc", bufs=1, space="PSUM"))

        def const_tile(name, dram, shape, dt):
            t = consts.tile(shape, dt, tag=name)
            nc.sync.dma_start(t[:], dram.ap())
            return t

        w1t = const_tile("w1", w1_d, [FJ, H], bf16)
        w2t = const_tile("w2", w2_d, [H, H], bf16)
        w3t = const_tile("w3", w3_d, [H, H], bf16)
        wzt = const_tile("wz", wz_d, [H, H], bf16)
        w4t = const_tile("w4", w4_d, [H, H], bf16)
        w5t = const_tile("w5", w5_d, [H, H], bf16)
        ip_t = const_tile("ip", identp_d, [H, H], bf16)
        bv = const_tile("bv", bv_d, [H, 8], f32)

        def r3(ap, k):
            return ap.rearrange("p (k c) -> p k c", k=k)

        # square via DVE tensor_scalar pow: hits the 4x bf16 perf mode
        # (tensor_tensor mult only reaches ~1 elem/cycle). Inputs are
        # relu outputs (>= 0) so pow is safe.
        def square(dst, src):
            if USE_POW_SQUARE:
                nc.vector.tensor_scalar(dst, src, 2.0, None, ALU.pow)
            else:
                nc.vector.tensor_tensor(dst, src, src, ALU.mult)

        jets_off = 0
        ev_off = 0
        pending_tail = [None]
        for gi, (g, cap) in enumerate(groups):
            assert cap % 8 == 0 and cap <= 512
            JCg = g * cap
            prs = pairs_of(g)
            PG = len(prs)

            jt = jin.tile([FJ, JCg], bf16, tag="jt")
            nc.sync.dma_start(jt[:], jets_d.ap()[:, jets_off : jets_off + JCg])

            # ---- jets side: 4 layers, layer-major 1024-col chunks.
            def layer(dst, wt, src, width, evac):
                for c0 in range(0, width, 1024):
                    w = min(1024, width - c0)
                    ps = mm.tile([H, 1024], f32, tag="mm")
                    for s0 in range(0, w, 512):
                        sw = min(512, w - s0)
                        nc.tensor.matmul(ps[:, s0 : s0 + sw], wt[:],
                                         src[:, c0 + s0 : c0 + s0 + sw],
                                         start=True, stop=True)
                    evac(dst[:, c0 : c0 + w], ps[:, :w])

            def act_relu(bias_col):
                def f(dst, ps):
                    nc.scalar.activation(dst, ps, AF.Relu,
                                         bias=bv[:, bias_col : bias_col + 1])
                return f

            def act_copy(dst, ps):
                nc.scalar.copy(dst, ps)

            x1 = x12.tile([H, JCg], bf16, tag="x1")
            layer(x1, w1t, jt, JCg, act_relu(0))
            x2 = x12.tile([H, JCg], bf16, tag="x2")
            layer(x2, w2t, x1, JCg, act_relu(1))
            x = bigx.tile([H, JCg], bf16, tag="x")
            layer(x, w3t, x2, JCg, act_relu(2))
            z = bigx.tile([H, JCg], bf16, tag="z")
            layer(z, wzt, x, JCg, act_copy)

            agg6 = aggs.tile([H, 6 * cap], f32, tag="agg6")

            def max_tree(src_tile, nslices, dst_f32, tag, l1_pool=False,
                         cap=cap):
                m, cur, off = nslices, src_tile, 0
                if m == 1:
                    nc.vector.tensor_copy(dst_f32, src_tile[:, 0:cap])
                    return
                first = True
                while m > 1:
                    k2 = (m + 1) // 2
                    if k2 == 1:
                        nxt = dst_f32
                    else:
                        nxt = mxp.tile([H, k2 * cap], bf16, tag=tag)
                    a0 = cur[:, off : off + k2 * cap]
                    a1 = cur[:, off + (m - k2) * cap : off + m * cap]
                    nc.vector.tensor_tensor(r3(nxt[:, 0 : k2 * cap], k2),
                                            r3(a0, k2), r3(a1, k2), ALU.max)
                    cur, off, m = nxt, 0, k2
                    first = False

            # ---- pairs side: SC-slice chunks, lag-1 ysq+sums.
            SC = max(1, 2048 // cap)
            PCH = SC * cap
            y3 = bigy.tile([H, PG * cap], bf16, tag="y3")
            a_sy = acc.tile([H, cap], f32, tag="a_sy")
            a_qy = acc.tile([H, cap], f32, tag="a_qy")
            a_sx = acc.tile([H, cap], f32, tag="a_sx")
            a_qx = acc.tile([H, cap], f32, tag="a_qx")

            def ysq_sums(p0, k, cap=cap, PCH=PCH, y3=y3, PG=PG,
                         a_sy=a_sy, a_qy=a_qy):
                w = k * cap
                ysq = scr.tile([H, PCH], bf16, tag="ysq")
                square(ysq[:, :w], y3[:, p0 * cap : p0 * cap + w])
                for s in range(k):
                    sl = p0 + s
                    nc.tensor.matmul(a_sy[:, :cap], ip_t[:],
                                     y3[:, sl * cap : (sl + 1) * cap],
                                     start=(sl == 0), stop=(sl == PG - 1))
                for s in range(k):
                    sl = p0 + s
                    nc.tensor.matmul(a_qy[:, :cap], ip_t[:],
                                     ysq[:, s * cap : (s + 1) * cap],
                                     start=(sl == 0), stop=(sl == PG - 1))

            prev = None
            y3_i = [0]
            for p0 in range(0, PG, SC):
                k = min(SC, PG - p0)
                w = k * cap
                y1 = scr.tile([H, PCH], bf16, tag="y1")
                s = 0
                while s < k:
                    i = prs[p0 + s][0]
                    r = 1
                    while s + r < k and prs[p0 + s + r][0] == i:
                        r += 1
                    j0 = prs[p0 + s][1]
                    nc.vector.tensor_tensor(
                        r3(y1[:, s * cap : (s + r) * cap], r),
                        r3(z[:, i * cap : (i + 1) * cap], 1).broadcast_to(
                            [H, r, cap]),
                        r3(z[:, j0 * cap : (j0 + r) * cap], r),
                        ALU.add)
                    s += r
                nc.vector.tensor_scalar(y1[:, :w], y1[:, :w], bv[:, 3:4],
                                        0.0, ALU.add, ALU.max)
                y2 = scr.tile([H, PCH], bf16, tag="y2")
                for n0 in range(0, w, 1024):
                    cw = min(1024, w - n0)
                    ps = mm.tile([H, 1024], f32, tag="mm")
                    for s0 in range(0, cw, 512):
                        sw = min(512, cw - s0)
                        nc.tensor.matmul(ps[:, s0 : s0 + sw], w4t[:],
                                         y1[:, n0 + s0 : n0 + s0 + sw],
                                         start=True, stop=True)
                    nc.scalar.activation(y2[:, n0 : n0 + cw], ps[:, :cw],
                                         AF.Relu, bias=bv[:, 4:5])
                for n0 in range(0, w, 1024):
                    cw = min(1024, w - n0)
                    ps = mm.tile([H, 1024], f32, tag="mm")
                    for s0 in range(0, cw, 512):
                        sw = min(512, cw - s0)
                        nc.tensor.matmul(ps[:, s0 : s0 + sw], w5t[:],
                                         y2[:, n0 + s0 : n0 + s0 + sw],
                                         start=True, stop=True)
                    dst = y3[:, p0 * cap + n0 : p0 * cap + n0 + cw]
                    if y3_i[0] % 2 == 0:
                        nc.scalar.activation(dst, ps[:, :cw], AF.Relu,
                                             bias=bv[:, 5:6])
                    else:
                        nc.vector.tensor_scalar(dst, ps[:, :cw], bv[:, 5:6],
                                                0.0, ALU.add, ALU.max)
                    y3_i[0] += 1
                if prev is not None:
                    ysq_sums(*prev)
                prev = (p0, k)
                # The previous group's tail goes one chunk deep into this
                # group: V starts this group's y1 immediately (unblocking
                # PE and Scalar), and the tail fills V/PE slack while the
                # first chunk's matmuls run.
                if pending_tail[0] is not None:
                    pending_tail[0]()
                    pending_tail[0] = None

            # x-side square/max emitted after the pair chunks: the Vector
            # engine runs them while PE finishes the pair matmuls, instead
            # of delaying the first y1 (which PE waits on).
            xsq = bigx.tile([H, JCg], bf16, tag="xsq")
            square(xsq[:], x[:])
            max_tree(x, g, agg6[:, 2 * cap : 3 * cap], "xt")

            def tail(prev=prev, y3=y3, PG=PG, g=g, cap=cap, agg6=agg6,
                     a_sy=a_sy, a_qy=a_qy, a_sx=a_sx, a_qx=a_qx, xsq=xsq,
                     x=x, ev_off=ev_off, max_tree=max_tree,
                     ysq_sums=ysq_sums):
                ysq_sums(*prev)
                for s in range(g):
                    nc.tensor.matmul(a_sx[:, :cap], ip_t[:],
                                     x[:, s * cap : (s + 1) * cap],
                                     start=(s == 0), stop=(s == g - 1))
                for s in range(g):
                    nc.tensor.matmul(a_qx[:, :cap], ip_t[:],
                                     xsq[:, s * cap : (s + 1) * cap],
                                     start=(s == 0), stop=(s == g - 1))
                max_tree(y3, PG, agg6[:, 5 * cap : 6 * cap], "yt",
                         l1_pool=True)
                nc.scalar.copy(agg6[:, 0:cap], a_sx[:, :cap])
                nc.scalar.copy(agg6[:, cap : 2 * cap], a_qx[:, :cap])
                nc.scalar.copy(agg6[:, 3 * cap : 4 * cap], a_sy[:, :cap])
                nc.scalar.copy(agg6[:, 4 * cap : 5 * cap], a_qy[:, :cap])
                nc.sync.dma_start(
                    out6_d.ap()[:, 6 * ev_off : 6 * ev_off + 6 * cap],
                    agg6[:])

            pending_tail[0] = tail

            jets_off += JCg
            ev_off += cap
        pending_tail[0]()

    nc.compile()
    return nc


# ---------------- host-side math ----------------

BN_EPS = 1e-3


def fold_params(inp):
    """Fold normalization + BN into per-layer (W, b). All numpy fp32."""
    mean_j = np.asarray(inp["mean_jets"], np.float32)
    std_j = np.asarray(inp["std_jets"], np.float32)
    w1f = np.asarray(inp["w1_first"], np.float32)
    w1r = np.asarray(inp["w1_rest"], np.float32)
    bn1 = np.asarray(inp["bn1"], np.float32)  # [3,4,H]: gamma, beta, mean, var
    w2f = np.asarray(inp["w2_first"], np.float32)
    w2r = np.asarray(inp["w2_rest"], np.float32)
    bn2 = np.asarray(inp["bn2"], np.float32)

    def bn_sb(row):
        gm, bt, mu, vv = row[0], row[1], row[2], row[3]
        s = gm / np.sqrt(vv + BN_EPS)
        return s.astype(np.float32), (bt - mu * s).astype(np.float32)

    s11, t11 = bn_sb(bn1[0]); s12, t12 = bn_sb(bn1[1]); s13, t13 = bn_sb(bn1[2])
    s21, t21 = bn_sb(bn2[0]); s22, t22 = bn_sb(bn2[1]); s23, t23 = bn_sb(bn2[2])

    A = w1f / std_j[:, None]
    c = -(mean_j / std_j) @ w1f
    return dict(
        W1=A * s11[None, :], b1=c * s11 + t11,
        W2=w1r[0] * s12[None, :], b2=t12,
        W3=w1r[1] * s13[None, :], b3=t13,
        Wz=w2f * s21[None, :], bz=t21,
        W4=w2r[0] * s22[None, :], b4=t22,
        W5=w2r[1] * s23[None, :], b5=t23,
    )


# ---------------- full kernel entry point ----------------

N_CORES = 8

_cache = {}
_TRACE = [False]
_LAST_RESULT = [None]


def _get_program(groups_key):
    if groups_key not in _cache:
        _cache[groups_key] = build_program(list(groups_key))
    return _cache[groups_key]


def _np_dt(dt):
    return mybir.dt.np(dt)


def _plan(n):
    """Returns (groups, slots): groups = [(g, cap)], slots[c][gi] =
    (padded index array, real count) for core c, group gi."""
    gs = []
    idx_by_g = {}
    for g in range(2, 11):
        idx = np.nonzero(n == g)[0]
        if len(idx):
            gs.append(g)
            idx_by_g[g] = idx
    stray = np.nonzero((n < 2) | (n > 10))[0]
    if len(stray):
        if not gs:
            gs.append(2)
            idx_by_g[2] = stray
        else:
            idx_by_g[gs[-1]] = np.concatenate([idx_by_g[gs[-1]], stray])
    # Interleave big/small groups so a small group's serial jets chain
    # overlaps a big group's long pair phase: [10, 2, 9, 3, 8, 4, ...]
    desc = sorted(gs, key=lambda g: -g)
    inter = []
    lo, hi = 0, len(desc) - 1
    while lo <= hi:
        inter.append(desc[lo]); lo += 1
        if lo <= hi:
            inter.append(desc[hi]); hi -= 1
    gs = inter
    groups = []
    slots = [[] for _ in range(N_CORES)]
    for g in gs:
        idx = idx_by_g[g]
        per_core = [idx[c::N_CORES] for c in range(N_CORES)]
        mx = max(len(p) for p in per_core)
        cap = max(8, ((mx + 7) // 8) * 8)
        groups.append((g, cap))
        fill = idx[0]
        for c in range(N_CORES):
            p = per_core[c]
            pad = np.full(cap, p[0] if len(p) else fill, dtype=np.int64)
            pad[: len(p)] = p
            slots[c].append((pad, len(p)))
    return groups, slots


def _pack_jets(jets, groups, slots_c):
    cols = []
    for (g, cap), (ids, _cnt) in zip(groups, slots_c):
        ev = jets[ids][:, :g, :]  # [cap, g, 16]
        cols.append(np.ascontiguousarray(ev.transpose(2, 1, 0)).reshape(
            FJ, g * cap))
    return np.concatenate(cols, axis=1).astype(_np_dt(bf16), copy=False)


def kernel(**inputs):
    from concourse.bass_utils import run_bass_kernel_spmd

    jets = np.asarray(inputs["inputs_jets"], dtype=np.float32)
    B = jets.shape[0]
    mask = (jets != 0.0).any(-1)
    n = mask.sum(-1).astype(np.int64)
    # compact valid jets to the front (no-op for the standard generator)
    if not np.array_equal(mask, np.arange(jets.shape[1])[None, :] < n[:, None]):
        order = np.argsort(~mask, axis=1, kind="stable")
        jets = np.take_along_axis(jets, order[:, :, None], axis=1)

    P = fold_params(inputs)
    groups, slots = _plan(n)
    nc = _get_program(tuple(groups))

    bvec = np.zeros((H, 8), np.float32)
    for i, k in enumerate(["b1", "b2", "b3", "bz", "b4", "b5"]):
        bvec[:, i] = P[k]
    ident = np.eye(H, dtype=np.float32)
    bnp = _np_dt(bf16)
    common = {
        "w1": P["W1"].astype(bnp), "w2": P["W2"].astype(bnp),
        "w3": P["W3"].astype(bnp), "wz": P["Wz"].astype(bnp),
        "w4": P["W4"].astype(bnp), "w5": P["W5"].astype(bnp),
        "identp": ident.astype(bnp), "bvec": bvec,
    }
    in_maps = []
    for c in range(N_CORES):
        m = dict(common)
        m["jets"] = _pack_jets(jets, groups, slots[c])
        in_maps.append(m)

    res = run_bass_kernel_spmd(nc, in_maps, core_ids=list(range(N_CORES)),
                               trace=_TRACE[0])
    _LAST_RESULT[0] = res

    agg_x = np.empty((B, 4 * H), np.float32)
    agg_y = np.empty((B, 4 * H), np.float32)
    for c in range(N_CORES):
        o6 = res.results[c]["out6"]  # [H, 6*EC] f32
        ev_off = 0
        for (g, cap), (ids, cnt) in zip(groups, slots[c]):
            blk = o6[:, 6 * ev_off : 6 * ev_off + 6 * cap]
            sx = blk[:, 0:cap].T[:cnt]
            qx = blk[:, cap : 2 * cap].T[:cnt]
            mx = blk[:, 2 * cap : 3 * cap].T[:cnt]
            sy = blk[:, 3 * cap : 4 * cap].T[:cnt]
            qy = blk[:, 4 * cap : 5 * cap].T[:cnt]
            my = blk[:, 5 * cap : 6 * cap].T[:cnt]
            ii = ids[:cnt]
            mean_x = sx / g
            mean_y = sy / (g * (g - 1) // 2)
            agg_x[ii] = np.concatenate(
                [sx, mx, mean_x, qx / g - mean_x * mean_x], axis=1)
            agg_y[ii] = np.concatenate(
                [sy, my, mean_y, qy / (g * (g - 1) // 2) - mean_y * mean_y],
                axis=1)
            ev_off += cap
    return agg_x, agg_y
